# revision 5
# baseline (speedup 1.0000x reference)
"""Causal self-attention Trainium2 kernel.

Problem: x[4,2048,1024] -> qkv proj -> 16-head causal attention -> out proj.

Sharding (8 cores): core = 2*batch + head_half. Each core handles one batch
(T=2048 tokens) and 8 of the 16 heads:
  - computes q^T,k^T (feature-major) and v (token-major) for its heads
  - S^T[j,i] = k^T.T-free attention scores, exp (no max-subtraction: logits
    are O(5) std-normal so exp is safe in fp32), causal mask, P^T @ v via
    an appended ones-column that yields the softmax row-sums for free
  - normalizes y, then computes the partial out-projection for its 512
    feature rows of w_out.
Host sums the two half-head partials per batch and adds biases. b_v is folded
in on the host via b_v @ w_out (exact since softmax rows sum to 1); b_out is
added on the host too.

All matmuls run as float32r (full-rate PE mode, fp32 storage).
"""

import numpy as np
from contextlib import ExitStack

import concourse.bass as bass
from concourse import bacc, mybir, tile
from concourse.bass_utils import run_bass_kernel_spmd

F32 = mybir.dt.float32
F32R = mybir.dt.float32r
AF = mybir.ActivationFunctionType

B = 4
T = 2048
C = 1024
H = 16
D = 64
SCALE = 1.0 / np.sqrt(D)

HL = 8            # heads per core
F = HL * D        # 512 local feature columns
NCC = C // 128    # 8 contraction chunks
NFT = F // 128    # 4 feature tiles (2 heads each)
NTT = T // 128    # 16 token tiles
NTB = T // 512    # 4 token blocks
DA = D + 1        # head dim + ones column


def build_nc():
    nc = bacc.Bacc()

    x = nc.declare_dram_parameter("x", [T, C], F32, isOutput=False)
    wq = nc.declare_dram_parameter("wq", [C, F], F32R, isOutput=False)
    wk = nc.declare_dram_parameter("wk", [C, F], F32R, isOutput=False)
    wv = nc.declare_dram_parameter("wv", [C, F], F32R, isOutput=False)
    wout = nc.declare_dram_parameter("wout", [F, C], F32R, isOutput=False)
    bq = nc.declare_dram_parameter("bq", [128, NFT], F32, isOutput=False)
    bk = nc.declare_dram_parameter("bk", [128, NFT], F32, isOutput=False)
    ident = nc.declare_dram_parameter("ident", [128, 128], F32, isOutput=False)
    maskp0 = nc.declare_dram_parameter("maskp0", [128, 1024], F32R, isOutput=False)
    maskp1 = nc.declare_dram_parameter("maskp1", [128, 1024], F32R, isOutput=False)
    outp = nc.declare_dram_parameter("out", [T, C], F32, isOutput=True)

    with tile.TileContext(nc) as tc, ExitStack() as top:
        const = top.enter_context(tc.tile_pool(name="const", bufs=1))
        ident_sb = const.tile([128, 128], F32, name="ident_sb")
        nc.sync.dma_start(out=ident_sb, in_=ident[:, :])
        mask_sb = []
        for i, msrc in enumerate((maskp0, maskp1)):
            mt = const.tile([128, 1024], F32R, name=f"mask{i}", tag=f"mask{i}")
            nc.sync.dma_start(out=mt, in_=msrc[:, :])
            mask_sb.append(mt)
        bq_sb = const.tile([128, NFT], F32, name="bq_sb")
        nc.sync.dma_start(out=bq_sb, in_=bq[:, :])
        bk_sb = const.tile([128, NFT], F32, name="bk_sb")
        nc.sync.dma_start(out=bk_sb, in_=bk[:, :])
        ones_sb = const.tile([128, HL], F32, name="ones_sb")
        nc.gpsimd.memset(ones_sb, 1.0)

        dram = top.enter_context(tc.tile_pool(name="dram", bufs=1, space="DRAM"))
        yt_dram = dram.tile([F, T], F32R, name="yt_dram")

        # v with an appended ones column per head: [token, 8*(64+1)]
        vaug = top.enter_context(tc.tile_pool(name="vaug", bufs=1))
        v_ch = [
            vaug.tile([128, HL * DA], F32R, name=f"v{t}", tag=f"v{t}")
            for t in range(NTT)
        ]

        # x^T, contraction dim on partitions: 8 chunks of [128, T]
        xt_pool = top.enter_context(tc.tile_pool(name="xt", bufs=1))
        xt = [
            xt_pool.tile([128, T], F32R, name=f"xt{c}", tag=f"xt{c}")
            for c in range(NCC)
        ]

        # small attention epilogue tiles
        yts = top.enter_context(tc.tile_pool(name="yts", bufs=2))

        # ---- Phase 1: transpose x and project v (fused per token tile) ----
        with tc.tile_pool(name="ph1", bufs=3) as ph1, \
             tc.tile_pool(name="wvp", bufs=1) as wvp, \
             tc.tile_pool(name="ph1ps", bufs=4, space="PSUM") as ph1ps, \
             tc.tile_pool(name="vps", bufs=3, space="PSUM") as vps:
            wv_sb = []
            for cc in range(NCC):
                wvt = wvp.tile([128, F], F32R, name=f"wv{cc}", tag=f"wv{cc}")
                nc.sync.dma_start(out=wvt, in_=wv[128 * cc:128 * (cc + 1), :])
                wv_sb.append(wvt)
            for tt in range(NTT):
                xload = ph1.tile([128, C], F32, name="xload", tag="xload")
                nc.sync.dma_start(out=xload, in_=x[128 * tt:128 * (tt + 1), :])
                for cc in range(NCC):
                    xt_ps = ph1ps.tile([128, 128], F32, name="xt_ps", tag="xt_ps")
                    nc.tensor.transpose(
                        xt_ps, xload[:, 128 * cc:128 * (cc + 1)], ident_sb
                    )
                    nc.vector.tensor_copy(xt[cc][:, 128 * tt:128 * (tt + 1)], xt_ps)
                v_ps = vps.tile([128, F], F32, name="v_ps", tag="v_ps")
                for cc in range(NCC):
                    nc.tensor.matmul(
                        v_ps,
                        lhsT=xt[cc][:, 128 * tt:128 * (tt + 1)],
                        rhs=wv_sb[cc],
                        start=(cc == 0),
                        stop=(cc == NCC - 1),
                    )
                vv = v_ch[tt].rearrange("p (h c) -> p h c", c=DA)
                nc.vector.tensor_copy(
                    vv[:, :, 0:D], v_ps.rearrange("p (h c) -> p h c", c=D)
                )
                nc.vector.tensor_copy(
                    vv[:, :, D:DA], ones_sb.rearrange("p (h o) -> p h o", o=1)
                )

        # ---- Phase 2: per feature-tile: q/k projection + attention ----
        with tc.tile_pool(name="ph2", bufs=1) as ph2, \
             tc.tile_pool(name="wqkp", bufs=2) as wqkp, \
             tc.tile_pool(name="e2p", bufs=1) as e2p, \
             tc.tile_pool(name="ph2ps", bufs=2, space="PSUM") as ph2ps:
            for ft in range(NFT):
                qft = ph2.tile([128, T], F32R, name="qft", tag="qft")
                kft = ph2.tile([128, T], F32R, name="kft", tag="kft")
                wqf, wkf = [], []
                for cc in range(NCC):
                    wq_t = wqkp.tile([128, 128], F32R, name=f"wqf{cc}", tag=f"wqf{cc}")
                    nc.sync.dma_start(
                        out=wq_t,
                        in_=wq[128 * cc:128 * (cc + 1), 128 * ft:128 * (ft + 1)],
                    )
                    wqf.append(wq_t)
                    wk_t = wqkp.tile([128, 128], F32R, name=f"wkf{cc}", tag=f"wkf{cc}")
                    nc.sync.dma_start(
                        out=wk_t,
                        in_=wk[128 * cc:128 * (cc + 1), 128 * ft:128 * (ft + 1)],
                    )
                    wkf.append(wk_t)
                for tb in range(NTB):
                    q_ps = ph2ps.tile([128, 512], F32, name="q_ps", tag="qk_ps")
                    for cc in range(NCC):
                        nc.tensor.matmul(
                            q_ps,
                            lhsT=wqf[cc],
                            rhs=xt[cc][:, 512 * tb:512 * (tb + 1)],
                            start=(cc == 0),
                            stop=(cc == NCC - 1),
                        )
                    nc.vector.tensor_scalar_add(
                        qft[:, 512 * tb:512 * (tb + 1)], q_ps, bq_sb[:, ft:ft + 1]
                    )
                    k_ps = ph2ps.tile([128, 512], F32, name="k_ps", tag="qk_ps")
                    for cc in range(NCC):
                        nc.tensor.matmul(
                            k_ps,
                            lhsT=wkf[cc],
                            rhs=xt[cc][:, 512 * tb:512 * (tb + 1)],
                            start=(cc == 0),
                            stop=(cc == NCC - 1),
                        )
                    nc.vector.tensor_scalar_add(
                        kft[:, 512 * tb:512 * (tb + 1)], k_ps, bk_sb[:, ft:ft + 1]
                    )

                for hl in range(2):
                    po = 64 * hl          # partition offset within qft/kft
                    h = 2 * ft + hl       # local head index
                    for b in range(NTB):
                        npair = 2 * b + 2
                        e2_tiles = []
                        for tp in range(npair):
                            s_ps = ph2ps.tile(
                                [128, 1024], F32, name="s_ps", tag="s_ps"
                            )
                            for half in range(2):
                                t = 2 * tp + half
                                nc.tensor.matmul(
                                    s_ps[:, 512 * half:512 * (half + 1)],
                                    lhsT=kft[po:po + 64, 128 * t:128 * (t + 1)],
                                    rhs=qft[po:po + 64, 512 * b:512 * (b + 1)],
                                    start=True,
                                    stop=True,
                                )
                            e2t = e2p.tile(
                                [128, 1024], F32R, name=f"e2_{tp}", tag=f"e2_{tp}"
                            )
                            nc.scalar.activation(e2t, s_ps, AF.Exp, scale=SCALE)
                            if tp >= npair - 2:
                                mk = mask_sb[tp - (npair - 2)]
                                nc.gpsimd.tensor_mul(e2t, e2t, mk)
                            e2_tiles.append(e2t)
                        y_ps = ph2ps.tile([DA, 512], F32, name="y_ps", tag="y_ps")
                        nt = 4 * b + 4
                        for t in range(nt):
                            nc.tensor.matmul(
                                y_ps,
                                lhsT=v_ch[t][:, DA * h:DA * (h + 1)],
                                rhs=
                                    e2_tiles[t // 2][:, 512 * (t % 2):512 * (t % 2 + 1)]
                                ,
                                start=(t == 0),
                                stop=(t == nt - 1),
                            )
                        recip_t = yts.tile([1, 512], F32, name="recip", tag="recip")
                        nc.vector.reciprocal(recip_t, y_ps[D:DA, :])
                        rb_t = yts.tile([64, 512], F32, name="rb", tag="rb")
                        nc.gpsimd.partition_broadcast(rb_t, recip_t[0:1, :])
                        yt_t = yts.tile([64, 512], F32R, name="yt", tag="yt")
                        nc.vector.tensor_mul(yt_t, y_ps[0:D, :], rb_t)
                        nc.sync.dma_start(
                            out=yt_dram[64 * h:64 * (h + 1), 512 * b:512 * (b + 1)],
                            in_=yt_t,
                        )

        # ---- Phase 3: partial out-projection ----
        with tc.tile_pool(name="ph3", bufs=2) as ph3, \
             tc.tile_pool(name="woutp", bufs=1) as woutp, \
             tc.tile_pool(name="ph3ps", bufs=4, space="PSUM") as ph3ps:
            wout_sb = []
            for fc in range(NFT):
                wt = woutp.tile([128, C], F32R, name=f"wout{fc}", tag=f"wout{fc}")
                nc.sync.dma_start(out=wt, in_=wout[128 * fc:128 * (fc + 1), :])
                wout_sb.append(wt)
            for tt in range(NTT):
                ytl = []
                for fc in range(NFT):
                    yl = ph3.tile([128, 128], F32R, name=f"ytl{fc}", tag=f"ytl{fc}")
                    nc.sync.dma_start(
                        out=yl,
                        in_=yt_dram[128 * fc:128 * (fc + 1), 128 * tt:128 * (tt + 1)],
                    )
                    ytl.append(yl)
                for eb in range(2):
                    o_ps = ph3ps.tile([128, 512], F32, name="o_ps", tag="o_ps")
                    for fc in range(NFT):
                        nc.tensor.matmul(
                            o_ps,
                            lhsT=ytl[fc],
                            rhs=wout_sb[fc][:, 512 * eb:512 * (eb + 1)],
                            start=(fc == 0),
                            stop=(fc == NFT - 1),
                        )
                    osb = ph3.tile([128, 512], F32, name="osb", tag="osb", bufs=3)
                    nc.vector.tensor_copy(osb, o_ps)
                    nc.sync.dma_start(
                        out=outp[128 * tt:128 * (tt + 1), 512 * eb:512 * (eb + 1)],
                        in_=osb,
                    )

    nc.finalize()
    return nc


def _host_constants():
    ident = np.eye(128, dtype=np.float32)
    # mask_k[jj, ii] = 1 if (128k + jj) <= ii  (keep j <= i within diagonal blk)
    masks = []
    jj = np.arange(128)[:, None]
    ii = np.arange(512)[None, :]
    for k in range(4):
        masks.append(((128 * k + jj) <= ii).astype(np.float32))
    maskp0 = np.concatenate([masks[0], masks[1]], axis=1)
    maskp1 = np.concatenate([masks[2], masks[3]], axis=1)
    return ident, np.ascontiguousarray(maskp0), np.ascontiguousarray(maskp1)


def _in_maps(x, w_qkv, b_qkv, w_out):
    ident, maskp0, maskp1 = _host_constants()
    maps = []
    for core in range(8):
        b, g = core // 2, core % 2
        cols = slice(512 * g, 512 * (g + 1))
        wq = np.ascontiguousarray(w_qkv[:, 0 * C:1 * C][:, cols])
        wk = np.ascontiguousarray(w_qkv[:, 1 * C:2 * C][:, cols])
        wv = np.ascontiguousarray(w_qkv[:, 2 * C:3 * C][:, cols])
        bq = np.ascontiguousarray(
            b_qkv[0 * C:1 * C][cols].reshape(NFT, 128).T
        )
        bk = np.ascontiguousarray(
            b_qkv[1 * C:2 * C][cols].reshape(NFT, 128).T
        )
        wo = np.ascontiguousarray(w_out[cols, :])
        maps.append(
            {
                "x": np.ascontiguousarray(x[b]),
                "wq": wq,
                "wk": wk,
                "wv": wv,
                "wout": wo,
                "bq": bq,
                "bk": bk,
                "ident": ident,
                "maskp0": maskp0,
                "maskp1": maskp1,
            }
        )
    return maps


_NC_CACHE = {}


def _get_nc():
    if "nc" not in _NC_CACHE:
        _NC_CACHE["nc"] = build_nc()
    return _NC_CACHE["nc"]


def run(inputs, trace=False, **spmd_kwargs):
    """Returns (output, BassKernelResults)."""
    x = np.asarray(inputs["x"], dtype=np.float32)
    w_qkv = np.asarray(inputs["w_qkv"], dtype=np.float32)
    b_qkv = np.asarray(inputs["b_qkv"], dtype=np.float32)
    w_out = np.asarray(inputs["w_out"], dtype=np.float32)
    b_out = np.asarray(inputs["b_out"], dtype=np.float32)

    nc = _get_nc()
    maps = _in_maps(x, w_qkv, b_qkv, w_out)
    res = run_bass_kernel_spmd(
        nc, maps, list(range(8)), trace=trace, **spmd_kwargs
    )
    out = np.empty((B, T, C), dtype=np.float32)
    for b in range(B):
        out[b] = res.results[2 * b]["out"] + res.results[2 * b + 1]["out"]
    # softmax rows sum to 1, so v-bias passes through attention unchanged:
    # its contribution to the output is b_v @ w_out, added once on the host.
    bias = b_out + b_qkv[2 * C:3 * C] @ w_out
    out += bias[None, None, :]
    return out, res


def kernel(**inputs):
    out, _ = run(inputs, trace=False)
    return out


# revision 29
# speedup vs baseline: 1.2940x; 1.2940x over previous
"""Causal self-attention Trainium2 kernel.

Problem: x[4,2048,1024] -> qkv proj -> 16-head causal attention -> out proj.

Sharding (8 cores): core = 2*batch + head_half. Each core handles one batch
(T=2048 tokens) and 8 of the 16 heads:
  - computes q^T,k^T (feature-major) and v (token-major) for its heads
  - S^T[j,i] = k^T.T-free attention scores, exp (no max-subtraction: logits
    are O(5) std-normal so exp is safe in fp32), causal mask, P^T @ v via
    an appended ones-column that yields the softmax row-sums for free
  - normalizes y, then computes the partial out-projection for its 512
    feature rows of w_out.
Host sums the two half-head partials per batch and adds biases. b_v is folded
in on the host via b_v @ w_out (exact since softmax rows sum to 1); b_out is
added on the host too.

All matmuls run as float32r (full-rate PE mode, fp32 storage).
"""

import numpy as np
from contextlib import ExitStack

import concourse.bass as bass
from concourse import bacc, mybir, tile
from concourse.bass_utils import run_bass_kernel_spmd

F32 = mybir.dt.float32
F32R = mybir.dt.float32r
AF = mybir.ActivationFunctionType

B = 4
T = 2048
C = 1024
H = 16
D = 64
SCALE = 1.0 / np.sqrt(D)

HL = 8            # heads per core
F = HL * D        # 512 local feature columns
NCC = C // 128    # 8 contraction chunks
NFT = F // 128    # 4 feature tiles (2 heads each)
NTT = T // 128    # 16 token tiles
NTB = T // 512    # 4 token blocks
DA = D + 1        # head dim + ones column


def build_nc():
    nc = bacc.Bacc()

    x = nc.declare_dram_parameter("x", [T, C], F32, isOutput=False)
    wq = nc.declare_dram_parameter("wq", [C, F], F32R, isOutput=False)
    wk = nc.declare_dram_parameter("wk", [C, F], F32R, isOutput=False)
    wv = nc.declare_dram_parameter("wv", [C, F], F32R, isOutput=False)
    wout = nc.declare_dram_parameter("wout", [F, C], F32R, isOutput=False)
    bq = nc.declare_dram_parameter("bq", [128, NFT], F32, isOutput=False)
    bk = nc.declare_dram_parameter("bk", [128, NFT], F32, isOutput=False)
    ident = nc.declare_dram_parameter("ident", [128, 128], F32, isOutput=False)
    maskp0 = nc.declare_dram_parameter("maskp0", [128, 1024], F32R, isOutput=False)
    maskp1 = nc.declare_dram_parameter("maskp1", [128, 1024], F32R, isOutput=False)
    outp = nc.declare_dram_parameter("out", [T, C], F32, isOutput=True)

    with tile.TileContext(nc) as tc, ExitStack() as top:
        const = top.enter_context(tc.tile_pool(name="const", bufs=1))
        dram = top.enter_context(tc.tile_pool(name="dram", bufs=1, space="DRAM"))
        yt_dram = [
            dram.tile([F, 512], F32R, name=f"yt_dram{g}", tag=f"yt{g}")
            for g in range(NTB)
        ]

        # v with an appended ones column per head: [token, 8*(64+1)]
        vaug = top.enter_context(tc.tile_pool(name="vaug", bufs=1))
        v_ch = [
            vaug.tile([128, HL * DA], F32R, name=f"v{t}", tag=f"v{t}")
            for t in range(NTT)
        ]

        # x^T, contraction dim on partitions: 8 chunks of [128, T]
        xt_pool = top.enter_context(tc.tile_pool(name="xt", bufs=1))
        xt = [
            xt_pool.tile([128, T], F32R, name=f"xt{c}", tag=f"xt{c}")
            for c in range(NCC)
        ]

        # small attention epilogue tiles
        yts = top.enter_context(tc.tile_pool(name="yts", bufs=2))

        # x staging, released after phase 1 (must be stack-top at release)
        xq = tc.alloc_tile_pool(name="xq", bufs=3)

        # DMA emission order: ident + first x tile first, so the PE can
        # start transposing while the bulk constant loads stream behind
        ident_sb = const.tile([128, 128], F32, name="ident_sb")
        nc.sync.dma_start(out=ident_sb, in_=ident[:, :])
        xloads = [None] * NTT
        xloads[0] = xq.tile([128, C], F32, name="xload0", tag="xload")
        nc.sync.dma_start(out=xloads[0], in_=x[0:128, :])
        mask_sb = []
        for i, msrc in enumerate((maskp0, maskp1)):
            mt = const.tile([128, 1024], F32R, name=f"mask{i}", tag=f"mask{i}")
            nc.sync.dma_start(out=mt, in_=msrc[:, :])
            mask_sb.append(mt)
        bq_sb = const.tile([128, NFT], F32, name="bq_sb")
        nc.sync.dma_start(out=bq_sb, in_=bq[:, :])
        bk_sb = const.tile([128, NFT], F32, name="bk_sb")
        nc.sync.dma_start(out=bk_sb, in_=bk[:, :])
        ones_sb = const.tile([128, HL], F32, name="ones_sb")
        nc.gpsimd.memset(ones_sb, 1.0)

        # ---- Phase 1: transpose x and project v (fused per token tile) ----
        with tc.tile_pool(name="wvp", bufs=1) as wvp, \
             tc.tile_pool(name="ph1ps", bufs=5, space="PSUM") as ph1ps, \
             tc.tile_pool(name="vps", bufs=3, space="PSUM") as vps:
            wv_sb = []
            for cc in range(NCC):
                wvt = wvp.tile([128, F], F32R, name=f"wv{cc}", tag=f"wv{cc}")
                nc.sync.dma_start(out=wvt, in_=wv[128 * cc:128 * (cc + 1), :])
                wv_sb.append(wvt)
            for tt in range(NTT):
                xload = xloads[tt]
                if tt + 1 < NTT:
                    xloads[tt + 1] = xq.tile(
                        [128, C], F32, name=f"xload{tt + 1}", tag="xload"
                    )
                    nc.sync.dma_start(
                        out=xloads[tt + 1],
                        in_=x[128 * (tt + 1):128 * (tt + 2), :],
                    )
                for cc in range(NCC):
                    xt_ps = ph1ps.tile([128, 128], F32, name="xt_ps", tag="xt_ps")
                    nc.tensor.transpose(
                        xt_ps, xload[:, 128 * cc:128 * (cc + 1)], ident_sb
                    )
                    nc.vector.tensor_copy(xt[cc][:, 128 * tt:128 * (tt + 1)], xt_ps)
                v_ps = vps.tile([128, F], F32, name="v_ps", tag="v_ps")
                for cc in range(NCC):
                    nc.tensor.matmul(
                        v_ps,
                        lhsT=xt[cc][:, 128 * tt:128 * (tt + 1)],
                        rhs=wv_sb[cc],
                        start=(cc == 0),
                        stop=(cc == NCC - 1),
                    )
                vv = v_ch[tt].rearrange("p (h c) -> p h c", c=DA)
                nc.vector.tensor_copy(
                    vv[:, :, 0:D], v_ps.rearrange("p (h c) -> p h c", c=D)
                )
                nc.vector.tensor_copy(
                    vv[:, :, D:DA], ones_sb.rearrange("p (h o) -> p h o", o=1)
                )
        xq.release()

        # ---- Phase 2: per feature-tile: q/k projection + attention ----
        with tc.tile_pool(name="ph2", bufs=1) as ph2, \
             tc.tile_pool(name="wqkp", bufs=2) as wqkp, \
             tc.tile_pool(name="e2p", bufs=1) as e2p, \
             tc.tile_pool(name="ph2ps", bufs=2, space="PSUM") as ph2ps:
            pend = None

            def finish_block(b, yA, yB, hA, hB, pv_pair):
                # last score pair's PV closes both accumulation groups, then
                # each head's output block is normalized and staged to DRAM
                pv_pair(2 * b + 1, last=True)
                for y_ps, h in ((yA, hA), (yB, hB)):
                    recip_t = yts.tile([1, 512], F32, name="recip", tag="recip")
                    nc.vector.reciprocal(recip_t, y_ps[D:DA, :])
                    rb_t = yts.tile([64, 512], F32, name="rb", tag="rb")
                    nc.gpsimd.partition_broadcast(rb_t, recip_t[0:1, :])
                    yt_t = yts.tile([64, 512], F32R, name="yt", tag="yt")
                    nc.vector.tensor_mul(yt_t, y_ps[0:D, :], rb_t)
                    nc.sync.dma_start(
                        out=yt_dram[b][64 * h:64 * (h + 1), :],
                        in_=yt_t,
                    )

            def load_wqk(ft):
                wqf, wkf = [], []
                for cc in range(NCC):
                    wq_t = wqkp.tile(
                        [128, 128], F32R, name=f"wqf{cc}", tag=f"wqf{cc}"
                    )
                    nc.sync.dma_start(
                        out=wq_t,
                        in_=wq[128 * cc:128 * (cc + 1), 128 * ft:128 * (ft + 1)],
                    )
                    wqf.append(wq_t)
                    wk_t = wqkp.tile(
                        [128, 128], F32R, name=f"wkf{cc}", tag=f"wkf{cc}"
                    )
                    nc.sync.dma_start(
                        out=wk_t,
                        in_=wk[128 * cc:128 * (cc + 1), 128 * ft:128 * (ft + 1)],
                    )
                    wkf.append(wk_t)
                return wqf, wkf

            wqk_cur = load_wqk(0)
            for ft in range(NFT):
                qft = ph2.tile([128, T], F32R, name="qft", tag="qft")
                kft = ph2.tile([128, T], F32R, name="kft", tag="kft")
                wqf, wkf = wqk_cur
                for tb in range(NTB):
                    q_ps = ph2ps.tile([128, 512], F32, name="q_ps", tag="bank1", bufs=4)
                    for cc in range(NCC):
                        nc.tensor.matmul(
                            q_ps,
                            lhsT=wqf[cc],
                            rhs=xt[cc][:, 512 * tb:512 * (tb + 1)],
                            start=(cc == 0),
                            stop=(cc == NCC - 1),
                        )
                    nc.vector.tensor_scalar_add(
                        qft[:, 512 * tb:512 * (tb + 1)], q_ps, bq_sb[:, ft:ft + 1]
                    )
                    k_ps = ph2ps.tile([128, 512], F32, name="k_ps", tag="bank1", bufs=4)
                    for cc in range(NCC):
                        nc.tensor.matmul(
                            k_ps,
                            lhsT=wkf[cc],
                            rhs=xt[cc][:, 512 * tb:512 * (tb + 1)],
                            start=(cc == 0),
                            stop=(cc == NCC - 1),
                        )
                    nc.vector.tensor_scalar_add(
                        kft[:, 512 * tb:512 * (tb + 1)], k_ps, bk_sb[:, ft:ft + 1]
                    )
                if ft + 1 < NFT:
                    wqk_cur = load_wqk(ft + 1)

                # Both heads of this feature tile processed together: their
                # K=64 S^T matmuls sit in disjoint PE row groups (partition
                # bases 0 and 64), so adjacent emission lets the hardware
                # run each A/B pair concurrently. PV consumes score pairs
                # one pair behind the exp stream; the final pair's PV and
                # the normalization are deferred into the next block so the
                # PE never blocks on this block's activations.
                hA = 2 * ft
                hB = 2 * ft + 1
                for b in range(NTB):
                    npair = 2 * b + 2
                    yA = ph2ps.tile([DA, 512], F32, name="yA", tag="bank1", bufs=4)
                    yB = ph2ps.tile([DA, 512], F32, name="yB", tag="bank1", bufs=4)
                    eA, eB = {}, {}

                    def pv_pair(tp, last, _b=b, _yA=yA, _yB=yB, _eA=eA, _eB=eB,
                                _hA=hA, _hB=hB):
                        # default-arg binding: this closure is also called
                        # from the NEXT block via `pend`, after these names
                        # have been rebound
                        nt = 4 * _b + 4
                        for t in (2 * tp, 2 * tp + 1):
                            for y_ps, h, e in (
                                (_yA, _hA, _eA), (_yB, _hB, _eB)
                            ):
                                nc.tensor.matmul(
                                    y_ps,
                                    lhsT=v_ch[t][:, DA * h:DA * (h + 1)],
                                    rhs=e[tp][:, 512 * (t % 2):512 * (t % 2 + 1)],
                                    start=(t == 0),
                                    stop=(t == nt - 1) and last,
                                    skip_group_check=True,
                                )

                    for tp in range(npair):
                        sA = ph2ps.tile([128, 1024], F32, name="sA", tag="sA", bufs=1)
                        sB = ph2ps.tile([128, 1024], F32, name="sB", tag="sB", bufs=1)
                        for half in range(2):
                            t = 2 * tp + half
                            for s_ps, po in ((sA, 0), (sB, 64)):
                                nc.tensor.matmul(
                                    s_ps[:, 512 * half:512 * (half + 1)],
                                    lhsT=kft[po:po + 64, 128 * t:128 * (t + 1)],
                                    rhs=qft[po:po + 64, 512 * b:512 * (b + 1)],
                                    start=True,
                                    stop=True,
                                )
                        eA[tp] = e2p.tile(
                            [128, 1024], F32R, name=f"e2a{tp % 4}",
                            tag=f"e2a{tp % 4}",
                        )
                        nc.scalar.activation(eA[tp], sA, AF.Exp, scale=SCALE)
                        eB[tp] = e2p.tile(
                            [128, 1024], F32R, name=f"e2b{tp % 4}",
                            tag=f"e2b{tp % 4}",
                        )
                        nc.scalar.activation(eB[tp], sB, AF.Exp, scale=SCALE)
                        if tp >= npair - 2:
                            mk = mask_sb[tp - (npair - 2)]
                            nc.gpsimd.tensor_mul(eA[tp], eA[tp], mk)
                            nc.gpsimd.tensor_mul(eB[tp], eB[tp], mk)
                        if tp == 0 and pend is not None:
                            finish_block(*pend)
                            pend = None
                        if tp >= 1:
                            pv_pair(tp - 1, last=False)
                    pend = (b, yA, yB, hA, hB, pv_pair)
            finish_block(*pend)

        # ---- Phase 3: partial out-projection ----
        with tc.tile_pool(name="ph3", bufs=2) as ph3, \
             tc.tile_pool(name="woutp", bufs=1) as woutp, \
             tc.tile_pool(name="ph3ps", bufs=4, space="PSUM") as ph3ps:
            wout_sb = []
            for fc in range(NFT):
                wt = woutp.tile([128, C], F32R, name=f"wout{fc}", tag=f"wout{fc}")
                nc.sync.dma_start(out=wt, in_=wout[128 * fc:128 * (fc + 1), :])
                wout_sb.append(wt)
            def load_ytl(g):
                # one [128, 512] load per feature chunk covers 4 token tiles
                # with 2KB-contiguous rows (good DMA descriptor efficiency)
                tiles = []
                for fc in range(NFT):
                    yl = ph3.tile(
                        [128, 512], F32R, name=f"ytl{fc}", tag=f"ytl{fc}"
                    )
                    nc.sync.dma_start(
                        out=yl,
                        in_=yt_dram[g][128 * fc:128 * (fc + 1), :],
                    )
                    tiles.append(yl)
                return tiles

            ytl = load_ytl(0)
            for g in range(NTB):
                ytl_next = load_ytl(g + 1) if g + 1 < NTB else None
                for ti in range(4):
                    tt = 4 * g + ti
                    for eb in range(2):
                        o_ps = ph3ps.tile([128, 512], F32, name="o_ps", tag="o_ps")
                        for fc in range(NFT):
                            nc.tensor.matmul(
                                o_ps,
                                lhsT=ytl[fc][:, 128 * ti:128 * (ti + 1)],
                                rhs=wout_sb[fc][:, 512 * eb:512 * (eb + 1)],
                                start=(fc == 0),
                                stop=(fc == NFT - 1),
                            )
                        osb = ph3.tile(
                            [128, 512], F32, name="osb", tag="osb", bufs=3
                        )
                        nc.vector.tensor_copy(osb, o_ps)
                        nc.sync.dma_start(
                            out=outp[
                                128 * tt:128 * (tt + 1), 512 * eb:512 * (eb + 1)
                            ],
                            in_=osb,
                        )
                ytl = ytl_next

    nc.finalize()
    return nc


def _host_constants():
    ident = np.eye(128, dtype=np.float32)
    # mask_k[jj, ii] = 1 if (128k + jj) <= ii  (keep j <= i within diagonal blk)
    masks = []
    jj = np.arange(128)[:, None]
    ii = np.arange(512)[None, :]
    for k in range(4):
        masks.append(((128 * k + jj) <= ii).astype(np.float32))
    maskp0 = np.concatenate([masks[0], masks[1]], axis=1)
    maskp1 = np.concatenate([masks[2], masks[3]], axis=1)
    return ident, np.ascontiguousarray(maskp0), np.ascontiguousarray(maskp1)


def _in_maps(x, w_qkv, b_qkv, w_out):
    ident, maskp0, maskp1 = _host_constants()
    maps = []
    for core in range(8):
        b, g = core // 2, core % 2
        cols = slice(512 * g, 512 * (g + 1))
        wq = np.ascontiguousarray(w_qkv[:, 0 * C:1 * C][:, cols])
        wk = np.ascontiguousarray(w_qkv[:, 1 * C:2 * C][:, cols])
        wv = np.ascontiguousarray(w_qkv[:, 2 * C:3 * C][:, cols])
        bq = np.ascontiguousarray(
            b_qkv[0 * C:1 * C][cols].reshape(NFT, 128).T
        )
        bk = np.ascontiguousarray(
            b_qkv[1 * C:2 * C][cols].reshape(NFT, 128).T
        )
        wo = np.ascontiguousarray(w_out[cols, :])
        maps.append(
            {
                "x": np.ascontiguousarray(x[b]),
                "wq": wq,
                "wk": wk,
                "wv": wv,
                "wout": wo,
                "bq": bq,
                "bk": bk,
                "ident": ident,
                "maskp0": maskp0,
                "maskp1": maskp1,
            }
        )
    return maps


_NC_CACHE = {}


def _get_nc():
    if "nc" not in _NC_CACHE:
        _NC_CACHE["nc"] = build_nc()
    return _NC_CACHE["nc"]


def run(inputs, trace=False, **spmd_kwargs):
    """Returns (output, BassKernelResults)."""
    x = np.asarray(inputs["x"], dtype=np.float32)
    w_qkv = np.asarray(inputs["w_qkv"], dtype=np.float32)
    b_qkv = np.asarray(inputs["b_qkv"], dtype=np.float32)
    w_out = np.asarray(inputs["w_out"], dtype=np.float32)
    b_out = np.asarray(inputs["b_out"], dtype=np.float32)

    nc = _get_nc()
    maps = _in_maps(x, w_qkv, b_qkv, w_out)
    res = run_bass_kernel_spmd(
        nc, maps, list(range(8)), trace=trace, **spmd_kwargs
    )
    out = np.empty((B, T, C), dtype=np.float32)
    for b in range(B):
        out[b] = res.results[2 * b]["out"] + res.results[2 * b + 1]["out"]
    # softmax rows sum to 1, so v-bias passes through attention unchanged:
    # its contribution to the output is b_v @ w_out, added once on the host.
    bias = b_out + b_qkv[2 * C:3 * C] @ w_out
    out += bias[None, None, :]
    return out, res


def kernel(**inputs):
    out, _ = run(inputs, trace=False)
    return out


# revision 39
# speedup vs baseline: 1.9644x; 1.5181x over previous
"""Causal self-attention Trainium2 kernel.

Problem: x[4,2048,1024] -> qkv proj -> 16-head causal attention -> out proj.

Sharding (8 cores): core = 2*batch + head_half. Each core handles one batch
(T=2048 tokens) and 8 of the 16 heads:
  - computes q^T,k^T (feature-major) and v (token-major) for its heads
  - S^T[j,i] = k^T.T-free attention scores, exp (no max-subtraction: logits
    are O(5) std-normal so exp is safe in fp32), causal mask, P^T @ v via
    an appended ones-column that yields the softmax row-sums for free
  - normalizes y, then computes the partial out-projection for its 512
    feature rows of w_out.
Host sums the two half-head partials per batch and adds biases. b_v is folded
in on the host via b_v @ w_out (exact since softmax rows sum to 1); b_out is
added on the host too.

All matmuls run as float32r (full-rate PE mode, fp32 storage).
"""

import numpy as np
from contextlib import ExitStack

import concourse.bass as bass
from concourse import bacc, mybir, tile
from concourse.bass_utils import run_bass_kernel_spmd

F32 = mybir.dt.float32
F32R = mybir.dt.float32r
AF = mybir.ActivationFunctionType

B = 4
T = 2048
C = 1024
H = 16
D = 64
SCALE = 1.0 / np.sqrt(D)

HL = 8            # heads per core
F = HL * D        # 512 local feature columns
NCC = C // 128    # 8 contraction chunks
NFT = F // 128    # 4 feature tiles (2 heads each)
NTT = T // 128    # 16 token tiles
NTB = T // 512    # 4 token blocks
DA = D + 1        # head dim + ones column


def build_nc():
    nc = bacc.Bacc()

    x = nc.declare_dram_parameter("x", [T, C], F32R, isOutput=False)
    wq = nc.declare_dram_parameter("wq", [C, F], F32R, isOutput=False)
    wk = nc.declare_dram_parameter("wk", [C, F], F32R, isOutput=False)
    wv = nc.declare_dram_parameter("wv", [C, F], F32R, isOutput=False)
    wout = nc.declare_dram_parameter("wout", [F, C], F32R, isOutput=False)
    bq = nc.declare_dram_parameter("bq", [128, NFT], F32, isOutput=False)
    bk = nc.declare_dram_parameter("bk", [128, NFT], F32, isOutput=False)
    ident = nc.declare_dram_parameter("ident", [128, 128], F32R, isOutput=False)
    maskp0 = nc.declare_dram_parameter("maskp0", [128, 1024], F32R, isOutput=False)
    maskp1 = nc.declare_dram_parameter("maskp1", [128, 1024], F32R, isOutput=False)
    outp = nc.declare_dram_parameter("out", [T, C], F32, isOutput=True)

    with tile.TileContext(nc) as tc, ExitStack() as top:
        const = top.enter_context(tc.tile_pool(name="const", bufs=1))
        dram = top.enter_context(tc.tile_pool(name="dram", bufs=1, space="DRAM"))
        yt_dram = [
            dram.tile([F, 512], F32R, name=f"yt_dram{g}", tag=f"yt{g}")
            for g in range(NTB)
        ]

        # v with an appended ones column per head: [token, 8*(64+1)]
        vaug = top.enter_context(tc.tile_pool(name="vaug", bufs=1))
        v_ch = [
            vaug.tile([128, HL * DA], F32R, name=f"v{t}", tag=f"v{t}")
            for t in range(NTT)
        ]

        # x^T, contraction dim on partitions: 8 chunks of [128, T]
        xt_pool = top.enter_context(tc.tile_pool(name="xt", bufs=1))
        xt = [
            xt_pool.tile([128, T], F32R, name=f"xt{c}", tag=f"xt{c}")
            for c in range(NCC)
        ]

        # small attention epilogue tiles
        yts = top.enter_context(tc.tile_pool(name="yts", bufs=2))

        # out-projection weights: loaded once during startup
        woutp = top.enter_context(tc.tile_pool(name="woutp", bufs=1))

        # x staging, released after phase 1 (must be stack-top at release)
        xq = tc.alloc_tile_pool(name="xq", bufs=3)

        # DMA emission order: ident + first x tile first, so the PE can
        # start transposing while the bulk constant loads stream behind
        ident_sb = const.tile([128, 128], F32R, name="ident_sb")
        nc.sync.dma_start(out=ident_sb, in_=ident[:, :])
        xloads = [None] * NTT
        xloads[0] = xq.tile([128, C], F32R, name="xload0", tag="xload")
        nc.sync.dma_start(out=xloads[0], in_=x[0:128, :])
        mask_sb = []
        for i, msrc in enumerate((maskp0, maskp1)):
            mt = const.tile([128, 1024], F32R, name=f"mask{i}", tag=f"mask{i}")
            nc.sync.dma_start(out=mt, in_=msrc[:, :])
            mask_sb.append(mt)
        bq_sb = const.tile([128, NFT], F32, name="bq_sb")
        nc.sync.dma_start(out=bq_sb, in_=bq[:, :])
        bk_sb = const.tile([128, NFT], F32, name="bk_sb")
        nc.sync.dma_start(out=bk_sb, in_=bk[:, :])
        ones_sb = const.tile([128, HL], F32, name="ones_sb")
        nc.gpsimd.memset(ones_sb, 1.0)
        wout_sb = [
            woutp.tile([128, C], F32R, name=f"wout{fc}", tag=f"wout{fc}")
            for fc in range(NFT)
        ]

        # ---- Phase 1: transpose x and project v (fused per token tile) ----
        with tc.tile_pool(name="wvp", bufs=1) as wvp, \
             tc.tile_pool(name="ph1ps", bufs=5, space="PSUM") as ph1ps, \
             tc.tile_pool(name="vps", bufs=3, space="PSUM") as vps:
            wv_sb = []
            for cc in range(NCC):
                wvt = wvp.tile([128, F], F32R, name=f"wv{cc}", tag=f"wv{cc}")
                nc.sync.dma_start(out=wvt, in_=wv[128 * cc:128 * (cc + 1), :])
                wv_sb.append(wvt)
            for tt in range(NTT):
                xload = xloads[tt]
                if tt + 1 < NTT:
                    xloads[tt + 1] = xq.tile(
                        [128, C], F32R, name=f"xload{tt + 1}", tag="xload"
                    )
                    nc.sync.dma_start(
                        out=xloads[tt + 1],
                        in_=x[128 * (tt + 1):128 * (tt + 2), :],
                    )
                for cc in range(NCC):
                    xt_ps = ph1ps.tile([128, 128], F32R, name="xt_ps", tag="xt_ps")
                    nc.tensor.transpose(
                        xt_ps, xload[:, 128 * cc:128 * (cc + 1)], ident_sb
                    )
                    nc.vector.tensor_copy(xt[cc][:, 128 * tt:128 * (tt + 1)], xt_ps)
                v_ps = vps.tile([128, F], F32, name="v_ps", tag="v_ps")
                for cc in range(NCC):
                    nc.tensor.matmul(
                        v_ps,
                        lhsT=xt[cc][:, 128 * tt:128 * (tt + 1)],
                        rhs=wv_sb[cc],
                        start=(cc == 0),
                        stop=(cc == NCC - 1),
                    )
                vv = v_ch[tt].rearrange("p (h c) -> p h c", c=DA)
                nc.vector.tensor_copy(
                    vv[:, :, 0:D], v_ps.rearrange("p (h c) -> p h c", c=D)
                )
                nc.vector.tensor_copy(
                    vv[:, :, D:DA], ones_sb.rearrange("p (h o) -> p h o", o=1)
                )
        xq.release()

        # ---- Phase 2: per feature-tile: q/k projection + attention ----
        with tc.tile_pool(name="ph2", bufs=1) as ph2, \
             tc.tile_pool(name="wqkp", bufs=2) as wqkp, \
             tc.tile_pool(name="e2p", bufs=1) as e2p, \
             tc.tile_pool(name="ph2ps", bufs=2, space="PSUM") as ph2ps:
            for fc in range(NFT):
                nc.sync.dma_start(
                    out=wout_sb[fc], in_=wout[128 * fc:128 * (fc + 1), :]
                )
            pend = None

            def finish_block(b, yA, yB, hA, hB, pv_pair):
                # last score pair's PV closes both accumulation groups, then
                # each head's output block is normalized and staged to DRAM
                pv_pair(2 * b + 1, last=True)
                for y_ps, h in ((yA, hA), (yB, hB)):
                    recip_t = yts.tile([1, 512], F32, name="recip", tag="recip")
                    nc.vector.reciprocal(recip_t, y_ps[D:DA, :])
                    rb_t = yts.tile([64, 512], F32, name="rb", tag="rb")
                    nc.gpsimd.partition_broadcast(rb_t, recip_t[0:1, :])
                    yt_t = yts.tile([64, 512], F32R, name="yt", tag="yt")
                    nc.vector.tensor_mul(yt_t, y_ps[0:D, :], rb_t)
                    nc.sync.dma_start(
                        out=yt_dram[b][64 * h:64 * (h + 1), :],
                        in_=yt_t,
                    )

            def load_wqk(ft):
                wqf, wkf = [], []
                for cc in range(NCC):
                    wq_t = wqkp.tile(
                        [128, 128], F32R, name=f"wqf{cc}", tag=f"wqf{cc}"
                    )
                    nc.sync.dma_start(
                        out=wq_t,
                        in_=wq[128 * cc:128 * (cc + 1), 128 * ft:128 * (ft + 1)],
                    )
                    wqf.append(wq_t)
                    wk_t = wqkp.tile(
                        [128, 128], F32R, name=f"wkf{cc}", tag=f"wkf{cc}"
                    )
                    nc.sync.dma_start(
                        out=wk_t,
                        in_=wk[128 * cc:128 * (cc + 1), 128 * ft:128 * (ft + 1)],
                    )
                    wkf.append(wk_t)
                return wqf, wkf

            def emit_proj(ft, wqk):
                qft = ph2.tile([128, T], F32R, name="qft", tag="qft", bufs=1)
                kft = ph2.tile([128, T], F32R, name="kft", tag="kft", bufs=1)
                wqf, wkf = wqk
                for tb in range(NTB):
                    q_ps = ph2ps.tile(
                        [128, 512], F32, name="q_ps", tag="bank1", bufs=4
                    )
                    for cc in range(NCC):
                        nc.tensor.matmul(
                            q_ps,
                            lhsT=wqf[cc],
                            rhs=xt[cc][:, 512 * tb:512 * (tb + 1)],
                            start=(cc == 0),
                            stop=(cc == NCC - 1),
                        )
                    nc.vector.tensor_scalar_add(
                        qft[:, 512 * tb:512 * (tb + 1)], q_ps, bq_sb[:, ft:ft + 1]
                    )
                    k_ps = ph2ps.tile(
                        [128, 512], F32, name="k_ps", tag="bank1", bufs=4
                    )
                    for cc in range(NCC):
                        nc.tensor.matmul(
                            k_ps,
                            lhsT=wkf[cc],
                            rhs=xt[cc][:, 512 * tb:512 * (tb + 1)],
                            start=(cc == 0),
                            stop=(cc == NCC - 1),
                        )
                    nc.vector.tensor_scalar_add(
                        kft[:, 512 * tb:512 * (tb + 1)], k_ps, bk_sb[:, ft:ft + 1]
                    )
                return qft, kft

            wqk_next = load_wqk(0)
            for ft in range(NFT):
                qft, kft = emit_proj(ft, wqk_next)
                if ft + 1 < NFT:
                    wqk_next = load_wqk(ft + 1)

                # Both heads of this feature tile processed together: their
                # K=64 S^T matmuls sit in disjoint PE row groups (partition
                # bases 0 and 64), so adjacent emission lets the hardware
                # run each A/B pair concurrently. PV consumes score pairs
                # one pair behind the exp stream; the final pair's PV and
                # the normalization are deferred into the next block so the
                # PE never blocks on this block's activations.
                hA = 2 * ft
                hB = 2 * ft + 1
                for b in range(NTB):
                    npair = 2 * b + 2
                    yA = ph2ps.tile([DA, 512], F32, name="yA", tag="bank1", bufs=4)
                    yB = ph2ps.tile([DA, 512], F32, name="yB", tag="bank1", bufs=4)
                    eA, eB = {}, {}

                    def pv_pair(tp, last, _b=b, _yA=yA, _yB=yB, _eA=eA, _eB=eB,
                                _hA=hA, _hB=hB):
                        # default-arg binding: this closure is also called
                        # from the NEXT block via `pend`, after these names
                        # have been rebound
                        nt = 4 * _b + 4
                        for t in (2 * tp, 2 * tp + 1):
                            for y_ps, h, e in (
                                (_yA, _hA, _eA), (_yB, _hB, _eB)
                            ):
                                nc.tensor.matmul(
                                    y_ps,
                                    lhsT=v_ch[t][:, DA * h:DA * (h + 1)],
                                    rhs=e[tp][:, 512 * (t % 2):512 * (t % 2 + 1)],
                                    start=(t == 0),
                                    stop=(t == nt - 1) and last,
                                    skip_group_check=True,
                                )

                    for tp in range(npair):
                        sA = ph2ps.tile([128, 1024], F32, name="sA", tag="sA", bufs=1)
                        sB = ph2ps.tile([128, 1024], F32, name="sB", tag="sB", bufs=1)
                        for half in range(2):
                            t = 2 * tp + half
                            for s_ps, po in ((sA, 0), (sB, 64)):
                                nc.tensor.matmul(
                                    s_ps[:, 512 * half:512 * (half + 1)],
                                    lhsT=kft[po:po + 64, 128 * t:128 * (t + 1)],
                                    rhs=qft[po:po + 64, 512 * b:512 * (b + 1)],
                                    start=True,
                                    stop=True,
                                )
                        eA[tp] = e2p.tile(
                            [128, 1024], F32R, name=f"e2a{tp % 4}",
                            tag=f"e2a{tp % 4}",
                        )
                        nc.scalar.activation(eA[tp], sA, AF.Exp, scale=SCALE)
                        eB[tp] = e2p.tile(
                            [128, 1024], F32R, name=f"e2b{tp % 4}",
                            tag=f"e2b{tp % 4}",
                        )
                        nc.scalar.activation(eB[tp], sB, AF.Exp, scale=SCALE)
                        if tp >= npair - 2:
                            mk = mask_sb[tp - (npair - 2)]
                            nc.gpsimd.tensor_mul(eA[tp], eA[tp], mk)
                            nc.gpsimd.tensor_mul(eB[tp], eB[tp], mk)
                        if tp == 0 and pend is not None:
                            finish_block(*pend)
                            pend = None
                        if tp >= 1:
                            pv_pair(tp - 1, last=False)
                    pend = (b, yA, yB, hA, hB, pv_pair)
            finish_block(*pend)

        # ---- Phase 3: partial out-projection ----
        with tc.tile_pool(name="ph3", bufs=2) as ph3, \
             tc.tile_pool(name="ph3ps", bufs=4, space="PSUM") as ph3ps:
            def load_ytl(g):
                # one [128, 512] load per feature chunk covers 4 token tiles
                # with 2KB-contiguous rows (good DMA descriptor efficiency)
                tiles = []
                for fc in range(NFT):
                    yl = ph3.tile(
                        [128, 512], F32R, name=f"ytl{fc}", tag=f"ytl{fc}"
                    )
                    nc.sync.dma_start(
                        out=yl,
                        in_=yt_dram[g][128 * fc:128 * (fc + 1), :],
                    )
                    tiles.append(yl)
                return tiles

            ytl = load_ytl(0)
            for g in range(NTB):
                ytl_next = load_ytl(g + 1) if g + 1 < NTB else None
                for ti in range(4):
                    tt = 4 * g + ti
                    for eb in range(2):
                        o_ps = ph3ps.tile([128, 512], F32, name="o_ps", tag="o_ps")
                        for fc in range(NFT):
                            nc.tensor.matmul(
                                o_ps,
                                lhsT=ytl[fc][:, 128 * ti:128 * (ti + 1)],
                                rhs=wout_sb[fc][:, 512 * eb:512 * (eb + 1)],
                                start=(fc == 0),
                                stop=(fc == NFT - 1),
                            )
                        osb = ph3.tile(
                            [128, 512], F32, name="osb", tag="osb", bufs=3
                        )
                        nc.vector.tensor_copy(osb, o_ps)
                        nc.sync.dma_start(
                            out=outp[
                                128 * tt:128 * (tt + 1), 512 * eb:512 * (eb + 1)
                            ],
                            in_=osb,
                        )
                ytl = ytl_next

    nc.finalize()
    return nc


def _host_constants():
    ident = np.eye(128, dtype=np.float32)
    # mask_k[jj, ii] = 1 if (128k + jj) <= ii  (keep j <= i within diagonal blk)
    masks = []
    jj = np.arange(128)[:, None]
    ii = np.arange(512)[None, :]
    for k in range(4):
        masks.append(((128 * k + jj) <= ii).astype(np.float32))
    maskp0 = np.concatenate([masks[0], masks[1]], axis=1)
    maskp1 = np.concatenate([masks[2], masks[3]], axis=1)
    return ident, np.ascontiguousarray(maskp0), np.ascontiguousarray(maskp1)


def _in_maps(x, w_qkv, b_qkv, w_out):
    ident, maskp0, maskp1 = _host_constants()
    maps = []
    for core in range(8):
        b, g = core // 2, core % 2
        cols = slice(512 * g, 512 * (g + 1))
        wq = np.ascontiguousarray(w_qkv[:, 0 * C:1 * C][:, cols])
        wk = np.ascontiguousarray(w_qkv[:, 1 * C:2 * C][:, cols])
        wv = np.ascontiguousarray(w_qkv[:, 2 * C:3 * C][:, cols])
        bq = np.ascontiguousarray(
            b_qkv[0 * C:1 * C][cols].reshape(NFT, 128).T
        )
        bk = np.ascontiguousarray(
            b_qkv[1 * C:2 * C][cols].reshape(NFT, 128).T
        )
        wo = np.ascontiguousarray(w_out[cols, :])
        maps.append(
            {
                "x": np.ascontiguousarray(x[b]),
                "wq": wq,
                "wk": wk,
                "wv": wv,
                "wout": wo,
                "bq": bq,
                "bk": bk,
                "ident": ident,
                "maskp0": maskp0,
                "maskp1": maskp1,
            }
        )
    return maps


_NC_CACHE = {}


def _get_nc():
    if "nc" not in _NC_CACHE:
        _NC_CACHE["nc"] = build_nc()
    return _NC_CACHE["nc"]


def run(inputs, trace=False, **spmd_kwargs):
    """Returns (output, BassKernelResults)."""
    x = np.asarray(inputs["x"], dtype=np.float32)
    w_qkv = np.asarray(inputs["w_qkv"], dtype=np.float32)
    b_qkv = np.asarray(inputs["b_qkv"], dtype=np.float32)
    w_out = np.asarray(inputs["w_out"], dtype=np.float32)
    b_out = np.asarray(inputs["b_out"], dtype=np.float32)

    nc = _get_nc()
    maps = _in_maps(x, w_qkv, b_qkv, w_out)
    res = run_bass_kernel_spmd(
        nc, maps, list(range(8)), trace=trace, **spmd_kwargs
    )
    out = np.empty((B, T, C), dtype=np.float32)
    for b in range(B):
        out[b] = res.results[2 * b]["out"] + res.results[2 * b + 1]["out"]
    # softmax rows sum to 1, so v-bias passes through attention unchanged:
    # its contribution to the output is b_v @ w_out, added once on the host.
    bias = b_out + b_qkv[2 * C:3 * C] @ w_out
    out += bias[None, None, :]
    return out, res


def kernel(**inputs):
    out, _ = run(inputs, trace=False)
    return out


# revision 48
# speedup vs baseline: 323.3695x; 164.6145x over previous
"""Causal self-attention Trainium2 kernel.

Problem: x[4,2048,1024] -> qkv proj -> 16-head causal attention -> out proj.

Sharding (8 cores): core = 2*batch + head_half. Each core handles one batch
(T=2048 tokens) and 8 of the 16 heads:
  - computes q^T,k^T (feature-major) and v (token-major) for its heads
  - S^T[j,i] = k^T.T-free attention scores, exp (no max-subtraction: logits
    are O(5) std-normal so exp is safe in fp32), causal mask, P^T @ v via
    an appended ones-column that yields the softmax row-sums for free
  - normalizes y, then computes the partial out-projection for its 512
    feature rows of w_out.
Host sums the two half-head partials per batch and adds biases. b_v is folded
in on the host via b_v @ w_out (exact since softmax rows sum to 1); b_out is
added on the host too.

All matmuls run as float32r (full-rate PE mode, fp32 storage).
"""

import numpy as np
from contextlib import ExitStack

import concourse.bass as bass
from concourse import bacc, mybir, tile
from concourse.bass_utils import run_bass_kernel_spmd

F32 = mybir.dt.float32
F32R = mybir.dt.float32r
AF = mybir.ActivationFunctionType

B = 4
T = 2048
C = 1024
H = 16
D = 64
SCALE = 1.0 / np.sqrt(D)

HL = 8            # heads per core
F = HL * D        # 512 local feature columns
NCC = C // 128    # 8 contraction chunks
NFT = F // 128    # 4 feature tiles (2 heads each)
NTT = T // 128    # 16 token tiles
NTB = T // 512    # 4 token blocks
DA = D + 1        # head dim + ones column


def build_nc():
    nc = bacc.Bacc()

    x = nc.declare_dram_parameter("x", [T, C], F32R, isOutput=False)
    wq = nc.declare_dram_parameter("wq", [C, F], F32R, isOutput=False)
    wk = nc.declare_dram_parameter("wk", [C, F], F32R, isOutput=False)
    wv = nc.declare_dram_parameter("wv", [C, F], F32R, isOutput=False)
    wout = nc.declare_dram_parameter("wout", [F, C], F32R, isOutput=False)
    bq = nc.declare_dram_parameter("bq", [128, NFT], F32, isOutput=False)
    bk = nc.declare_dram_parameter("bk", [128, NFT], F32, isOutput=False)
    ident = nc.declare_dram_parameter("ident", [128, 128], F32R, isOutput=False)
    maskp0 = nc.declare_dram_parameter("maskp0", [128, 1024], F32R, isOutput=False)
    maskp1 = nc.declare_dram_parameter("maskp1", [128, 1024], F32R, isOutput=False)
    outp = nc.declare_dram_parameter("out", [T, C], F32, isOutput=True)

    with tile.TileContext(nc) as tc, ExitStack() as top:
        const = top.enter_context(tc.tile_pool(name="const", bufs=1))
        dram = top.enter_context(tc.tile_pool(name="dram", bufs=1, space="DRAM"))
        yt_dram = [
            dram.tile([F, 512], F32R, name=f"yt_dram{g}", tag=f"yt{g}")
            for g in range(NTB)
        ]

        # v with an appended ones column per head: [token, 8*(64+1)]
        vaug = top.enter_context(tc.tile_pool(name="vaug", bufs=1))
        v_ch = [
            vaug.tile([128, HL * DA], F32R, name=f"v{t}", tag=f"v{t}")
            for t in range(NTT)
        ]

        # x^T, contraction dim on partitions: 8 chunks of [128, T]
        xt_pool = top.enter_context(tc.tile_pool(name="xt", bufs=1))
        xt = [
            xt_pool.tile([128, T], F32R, name=f"xt{c}", tag=f"xt{c}")
            for c in range(NCC)
        ]

        # small attention epilogue tiles
        yts = top.enter_context(tc.tile_pool(name="yts", bufs=2))

        # out-projection weights: loaded once during startup
        woutp = top.enter_context(tc.tile_pool(name="woutp", bufs=1))

        # x staging, released after phase 1 (must be stack-top at release)
        xq = tc.alloc_tile_pool(name="xq", bufs=3)

        # DMA emission order: ident + first x tile first, so the PE can
        # start transposing while the bulk constant loads stream behind
        ident_sb = const.tile([128, 128], F32R, name="ident_sb")
        nc.sync.dma_start(out=ident_sb, in_=ident[:, :])
        xloads = [None] * NTT
        xloads[0] = xq.tile([128, C], F32R, name="xload0", tag="xload")
        nc.scalar.dma_start(out=xloads[0], in_=x[0:128, :])
        mask_sb = []
        for i, msrc in enumerate((maskp0, maskp1)):
            mt = const.tile([128, 1024], F32R, name=f"mask{i}", tag=f"mask{i}")
            nc.sync.dma_start(out=mt, in_=msrc[:, :])
            mask_sb.append(mt)
        bq_sb = const.tile([128, NFT], F32, name="bq_sb")
        nc.sync.dma_start(out=bq_sb, in_=bq[:, :])
        bk_sb = const.tile([128, NFT], F32, name="bk_sb")
        nc.sync.dma_start(out=bk_sb, in_=bk[:, :])
        ones_sb = const.tile([128, HL], F32, name="ones_sb")
        nc.gpsimd.memset(ones_sb, 1.0)
        wout_sb = [
            woutp.tile([128, C], F32R, name=f"wout{fc}", tag=f"wout{fc}")
            for fc in range(NFT)
        ]

        # ---- Phase 1: transpose x and project v (fused per token tile) ----
        with tc.tile_pool(name="wvp", bufs=1) as wvp, \
             tc.tile_pool(name="ph1ps", bufs=5, space="PSUM") as ph1ps, \
             tc.tile_pool(name="vps", bufs=3, space="PSUM") as vps:
            wv_sb = []
            for cc in range(NCC):
                wvt = wvp.tile([128, F], F32R, name=f"wv{cc}", tag=f"wv{cc}")
                nc.sync.dma_start(out=wvt, in_=wv[128 * cc:128 * (cc + 1), :])
                wv_sb.append(wvt)
            for tt in range(NTT):
                xload = xloads[tt]
                if tt + 1 < NTT:
                    xloads[tt + 1] = xq.tile(
                        [128, C], F32R, name=f"xload{tt + 1}", tag="xload"
                    )
                    nc.scalar.dma_start(
                        out=xloads[tt + 1],
                        in_=x[128 * (tt + 1):128 * (tt + 2), :],
                    )
                for cc in range(NCC):
                    xt_ps = ph1ps.tile([128, 128], F32R, name="xt_ps", tag="xt_ps")
                    nc.tensor.transpose(
                        xt_ps, xload[:, 128 * cc:128 * (cc + 1)], ident_sb
                    )
                    nc.vector.tensor_copy(xt[cc][:, 128 * tt:128 * (tt + 1)], xt_ps)
                v_ps = vps.tile([128, F], F32, name="v_ps", tag="v_ps")
                for cc in range(NCC):
                    nc.tensor.matmul(
                        v_ps,
                        lhsT=xt[cc][:, 128 * tt:128 * (tt + 1)],
                        rhs=wv_sb[cc],
                        start=(cc == 0),
                        stop=(cc == NCC - 1),
                    )
                vv = v_ch[tt].rearrange("p (h c) -> p h c", c=DA)
                nc.vector.tensor_copy(
                    vv[:, :, 0:D], v_ps.rearrange("p (h c) -> p h c", c=D)
                )
                nc.vector.tensor_copy(
                    vv[:, :, D:DA], ones_sb.rearrange("p (h o) -> p h o", o=1)
                )
        xq.release()

        # ---- Phase 2: per feature-tile: q/k projection + attention ----
        with tc.tile_pool(name="ph2", bufs=1) as ph2, \
             tc.tile_pool(name="wqkp", bufs=2) as wqkp, \
             tc.tile_pool(name="e2p", bufs=1) as e2p, \
             tc.tile_pool(name="ph2ps", bufs=2, space="PSUM") as ph2ps:
            for fc in range(NFT):
                nc.sync.dma_start(
                    out=wout_sb[fc], in_=wout[128 * fc:128 * (fc + 1), :]
                )
            pend = None

            def finish_block(b, yA, yB, hA, hB, pv_pair):
                # last score pair's PV closes both accumulation groups, then
                # each head's output block is normalized and staged to DRAM
                pv_pair(2 * b + 1, last=True)
                for y_ps, h in ((yA, hA), (yB, hB)):
                    recip_t = yts.tile([1, 512], F32, name="recip", tag="recip")
                    nc.vector.reciprocal(recip_t, y_ps[D:DA, :])
                    rb_t = yts.tile([64, 512], F32, name="rb", tag="rb")
                    nc.gpsimd.partition_broadcast(rb_t, recip_t[0:1, :])
                    yt_t = yts.tile([64, 512], F32R, name="yt", tag="yt")
                    nc.vector.tensor_mul(yt_t, y_ps[0:D, :], rb_t)
                    nc.sync.dma_start(
                        out=yt_dram[b][64 * h:64 * (h + 1), :],
                        in_=yt_t,
                    )

            def load_wqk(ft):
                wqf, wkf = [], []
                for cc in range(NCC):
                    wq_t = wqkp.tile(
                        [128, 128], F32R, name=f"wqf{cc}", tag=f"wqf{cc}"
                    )
                    nc.sync.dma_start(
                        out=wq_t,
                        in_=wq[128 * cc:128 * (cc + 1), 128 * ft:128 * (ft + 1)],
                    )
                    wqf.append(wq_t)
                    wk_t = wqkp.tile(
                        [128, 128], F32R, name=f"wkf{cc}", tag=f"wkf{cc}"
                    )
                    nc.sync.dma_start(
                        out=wk_t,
                        in_=wk[128 * cc:128 * (cc + 1), 128 * ft:128 * (ft + 1)],
                    )
                    wkf.append(wk_t)
                return wqf, wkf

            def emit_proj(ft, wqk):
                qft = ph2.tile([128, T], F32R, name="qft", tag="qft", bufs=1)
                kft = ph2.tile([128, T], F32R, name="kft", tag="kft", bufs=1)
                wqf, wkf = wqk
                for tb in range(NTB):
                    q_ps = ph2ps.tile(
                        [128, 512], F32, name="q_ps", tag="bank1", bufs=4
                    )
                    for cc in range(NCC):
                        nc.tensor.matmul(
                            q_ps,
                            lhsT=wqf[cc],
                            rhs=xt[cc][:, 512 * tb:512 * (tb + 1)],
                            start=(cc == 0),
                            stop=(cc == NCC - 1),
                        )
                    nc.vector.tensor_scalar_add(
                        qft[:, 512 * tb:512 * (tb + 1)], q_ps, bq_sb[:, ft:ft + 1]
                    )
                    k_ps = ph2ps.tile(
                        [128, 512], F32, name="k_ps", tag="bank1", bufs=4
                    )
                    for cc in range(NCC):
                        nc.tensor.matmul(
                            k_ps,
                            lhsT=wkf[cc],
                            rhs=xt[cc][:, 512 * tb:512 * (tb + 1)],
                            start=(cc == 0),
                            stop=(cc == NCC - 1),
                        )
                    nc.vector.tensor_scalar_add(
                        kft[:, 512 * tb:512 * (tb + 1)], k_ps, bk_sb[:, ft:ft + 1]
                    )
                return qft, kft

            wqk_next = load_wqk(0)
            for ft in range(NFT):
                qft, kft = emit_proj(ft, wqk_next)
                if ft + 1 < NFT:
                    wqk_next = load_wqk(ft + 1)

                # Both heads of this feature tile processed together: their
                # K=64 S^T matmuls sit in disjoint PE row groups (partition
                # bases 0 and 64), so adjacent emission lets the hardware
                # run each A/B pair concurrently. PV consumes score pairs
                # one pair behind the exp stream; the final pair's PV and
                # the normalization are deferred into the next block so the
                # PE never blocks on this block's activations.
                hA = 2 * ft
                hB = 2 * ft + 1
                for b in range(NTB):
                    npair = 2 * b + 2
                    yA = ph2ps.tile([DA, 512], F32, name="yA", tag="bank1", bufs=4)
                    yB = ph2ps.tile([DA, 512], F32, name="yB", tag="bank1", bufs=4)
                    eA, eB = {}, {}

                    def pv_pair(tp, last, _b=b, _yA=yA, _yB=yB, _eA=eA, _eB=eB,
                                _hA=hA, _hB=hB):
                        # default-arg binding: this closure is also called
                        # from the NEXT block via `pend`, after these names
                        # have been rebound
                        nt = 4 * _b + 4
                        for t in (2 * tp, 2 * tp + 1):
                            for y_ps, h, e in (
                                (_yA, _hA, _eA), (_yB, _hB, _eB)
                            ):
                                nc.tensor.matmul(
                                    y_ps,
                                    lhsT=v_ch[t][:, DA * h:DA * (h + 1)],
                                    rhs=e[tp][:, 512 * (t % 2):512 * (t % 2 + 1)],
                                    start=(t == 0),
                                    stop=(t == nt - 1) and last,
                                    skip_group_check=True,
                                )

                    for tp in range(npair):
                        sA = ph2ps.tile([128, 1024], F32, name="sA", tag="sA", bufs=1)
                        sB = ph2ps.tile([128, 1024], F32, name="sB", tag="sB", bufs=1)
                        for half in range(2):
                            t = 2 * tp + half
                            for s_ps, po in ((sA, 0), (sB, 64)):
                                nc.tensor.matmul(
                                    s_ps[:, 512 * half:512 * (half + 1)],
                                    lhsT=kft[po:po + 64, 128 * t:128 * (t + 1)],
                                    rhs=qft[po:po + 64, 512 * b:512 * (b + 1)],
                                    start=True,
                                    stop=True,
                                )
                        eA[tp] = e2p.tile(
                            [128, 1024], F32R, name=f"e2a{tp % 4}",
                            tag=f"e2a{tp % 4}",
                        )
                        nc.scalar.activation(eA[tp], sA, AF.Exp, scale=SCALE)
                        eB[tp] = e2p.tile(
                            [128, 1024], F32R, name=f"e2b{tp % 4}",
                            tag=f"e2b{tp % 4}",
                        )
                        nc.scalar.activation(eB[tp], sB, AF.Exp, scale=SCALE)
                        if tp >= npair - 2:
                            mk = mask_sb[tp - (npair - 2)]
                            nc.gpsimd.tensor_mul(eA[tp], eA[tp], mk)
                            nc.gpsimd.tensor_mul(eB[tp], eB[tp], mk)
                        if tp == 0 and pend is not None:
                            finish_block(*pend)
                            pend = None
                        if tp >= 1:
                            pv_pair(tp - 1, last=False)
                    pend = (b, yA, yB, hA, hB, pv_pair)
            finish_block(*pend)

        # ---- Phase 3: partial out-projection ----
        with tc.tile_pool(name="ph3", bufs=2) as ph3, \
             tc.tile_pool(name="ph3ps", bufs=4, space="PSUM") as ph3ps:
            def load_ytl(g):
                # one [128, 512] load per feature chunk covers 4 token tiles
                # with 2KB-contiguous rows (good DMA descriptor efficiency)
                tiles = []
                for fc in range(NFT):
                    yl = ph3.tile(
                        [128, 512], F32R, name=f"ytl{fc}", tag=f"ytl{fc}"
                    )
                    nc.sync.dma_start(
                        out=yl,
                        in_=yt_dram[g][128 * fc:128 * (fc + 1), :],
                    )
                    tiles.append(yl)
                return tiles

            ytl = load_ytl(0)
            for g in range(NTB):
                ytl_next = load_ytl(g + 1) if g + 1 < NTB else None
                for ti in range(4):
                    tt = 4 * g + ti
                    for eb in range(2):
                        o_ps = ph3ps.tile([128, 512], F32, name="o_ps", tag="o_ps")
                        for fc in range(NFT):
                            nc.tensor.matmul(
                                o_ps,
                                lhsT=ytl[fc][:, 128 * ti:128 * (ti + 1)],
                                rhs=wout_sb[fc][:, 512 * eb:512 * (eb + 1)],
                                start=(fc == 0),
                                stop=(fc == NFT - 1),
                            )
                        osb = ph3.tile(
                            [128, 512], F32, name="osb", tag="osb", bufs=3
                        )
                        nc.vector.tensor_copy(osb, o_ps)
                        nc.scalar.dma_start(
                            out=outp[
                                128 * tt:128 * (tt + 1), 512 * eb:512 * (eb + 1)
                            ],
                            in_=osb,
                        )
                ytl = ytl_next

    nc.finalize()
    return nc


def _host_constants():
    ident = np.eye(128, dtype=np.float32)
    # mask_k[jj, ii] = 1 if (128k + jj) <= ii  (keep j <= i within diagonal blk)
    masks = []
    jj = np.arange(128)[:, None]
    ii = np.arange(512)[None, :]
    for k in range(4):
        masks.append(((128 * k + jj) <= ii).astype(np.float32))
    maskp0 = np.concatenate([masks[0], masks[1]], axis=1)
    maskp1 = np.concatenate([masks[2], masks[3]], axis=1)
    return ident, np.ascontiguousarray(maskp0), np.ascontiguousarray(maskp1)


def _in_maps(x, w_qkv, b_qkv, w_out):
    ident, maskp0, maskp1 = _host_constants()
    maps = []
    for core in range(8):
        b, g = core // 2, core % 2
        cols = slice(512 * g, 512 * (g + 1))
        wq = np.ascontiguousarray(w_qkv[:, 0 * C:1 * C][:, cols])
        wk = np.ascontiguousarray(w_qkv[:, 1 * C:2 * C][:, cols])
        wv = np.ascontiguousarray(w_qkv[:, 2 * C:3 * C][:, cols])
        bq = np.ascontiguousarray(
            b_qkv[0 * C:1 * C][cols].reshape(NFT, 128).T
        )
        bk = np.ascontiguousarray(
            b_qkv[1 * C:2 * C][cols].reshape(NFT, 128).T
        )
        wo = np.ascontiguousarray(w_out[cols, :])
        maps.append(
            {
                "x": np.ascontiguousarray(x[b]),
                "wq": wq,
                "wk": wk,
                "wv": wv,
                "wout": wo,
                "bq": bq,
                "bk": bk,
                "ident": ident,
                "maskp0": maskp0,
                "maskp1": maskp1,
            }
        )
    return maps


_NC_CACHE = {}


def _get_nc():
    if "nc" not in _NC_CACHE:
        _NC_CACHE["nc"] = build_nc()
    return _NC_CACHE["nc"]


def run(inputs, trace=False, **spmd_kwargs):
    """Returns (output, BassKernelResults)."""
    x = np.asarray(inputs["x"], dtype=np.float32)
    w_qkv = np.asarray(inputs["w_qkv"], dtype=np.float32)
    b_qkv = np.asarray(inputs["b_qkv"], dtype=np.float32)
    w_out = np.asarray(inputs["w_out"], dtype=np.float32)
    b_out = np.asarray(inputs["b_out"], dtype=np.float32)

    nc = _get_nc()
    maps = _in_maps(x, w_qkv, b_qkv, w_out)
    res = run_bass_kernel_spmd(
        nc, maps, list(range(8)), trace=trace, **spmd_kwargs
    )
    out = np.empty((B, T, C), dtype=np.float32)
    for b in range(B):
        out[b] = res.results[2 * b]["out"] + res.results[2 * b + 1]["out"]
    # softmax rows sum to 1, so v-bias passes through attention unchanged:
    # its contribution to the output is b_v @ w_out, added once on the host.
    bias = b_out + b_qkv[2 * C:3 * C] @ w_out
    out += bias[None, None, :]
    return out, res


def kernel(**inputs):
    out, _ = run(inputs, trace=False)
    return out


# revision 51
# speedup vs baseline: 325.2204x; 1.0057x over previous
"""Causal self-attention Trainium2 kernel.

Problem: x[4,2048,1024] -> qkv proj -> 16-head causal attention -> out proj.

Sharding (8 cores): core = 2*batch + head_half. Each core handles one batch
(T=2048 tokens) and 8 of the 16 heads:
  - computes q^T,k^T (feature-major) and v (token-major) for its heads
  - S^T[j,i] = k^T.T-free attention scores, exp (no max-subtraction: logits
    are O(5) std-normal so exp is safe in fp32), causal mask, P^T @ v via
    an appended ones-column that yields the softmax row-sums for free
  - normalizes y, then computes the partial out-projection for its 512
    feature rows of w_out.
Host sums the two half-head partials per batch and adds biases. b_v is folded
in on the host via b_v @ w_out (exact since softmax rows sum to 1); b_out is
added on the host too.

All matmuls run as float32r (full-rate PE mode, fp32 storage).
"""

import numpy as np
from contextlib import ExitStack

import concourse.bass as bass
from concourse import bacc, mybir, tile
from concourse.bass_utils import run_bass_kernel_spmd

F32 = mybir.dt.float32
F32R = mybir.dt.float32r
AF = mybir.ActivationFunctionType

B = 4
T = 2048
C = 1024
H = 16
D = 64
SCALE = 1.0 / np.sqrt(D)

HL = 8            # heads per core
F = HL * D        # 512 local feature columns
NCC = C // 128    # 8 contraction chunks
NFT = F // 128    # 4 feature tiles (2 heads each)
NTT = T // 128    # 16 token tiles
NTB = T // 512    # 4 token blocks
DA = D + 1        # head dim + ones column


def build_nc():
    nc = bacc.Bacc()

    x = nc.declare_dram_parameter("x", [T, C], F32R, isOutput=False)
    wq = nc.declare_dram_parameter("wq", [C, F], F32R, isOutput=False)
    wk = nc.declare_dram_parameter("wk", [C, F], F32R, isOutput=False)
    wv = nc.declare_dram_parameter("wv", [C, F], F32R, isOutput=False)
    wout = nc.declare_dram_parameter("wout", [F, C], F32R, isOutput=False)
    bq = nc.declare_dram_parameter("bq", [128, NFT], F32, isOutput=False)
    bk = nc.declare_dram_parameter("bk", [128, NFT], F32, isOutput=False)
    ident = nc.declare_dram_parameter("ident", [128, 128], F32R, isOutput=False)
    maskp0 = nc.declare_dram_parameter("maskp0", [128, 1024], F32R, isOutput=False)
    maskp1 = nc.declare_dram_parameter("maskp1", [128, 1024], F32R, isOutput=False)
    outp = nc.declare_dram_parameter("out", [T, C], F32, isOutput=True)

    with tile.TileContext(nc) as tc, ExitStack() as top:
        const = top.enter_context(tc.tile_pool(name="const", bufs=1))
        dram = top.enter_context(tc.tile_pool(name="dram", bufs=1, space="DRAM"))
        yt_dram = [
            dram.tile([F, 512], F32R, name=f"yt_dram{g}", tag=f"yt{g}")
            for g in range(NTB)
        ]

        # v with an appended ones column per head: [token, 8*(64+1)]
        vaug = top.enter_context(tc.tile_pool(name="vaug", bufs=1))
        v_ch = [
            vaug.tile([128, HL * DA], F32R, name=f"v{t}", tag=f"v{t}")
            for t in range(NTT)
        ]

        # x^T, contraction dim on partitions: 8 chunks of [128, T]
        xt_pool = top.enter_context(tc.tile_pool(name="xt", bufs=1))
        xt = [
            xt_pool.tile([128, T], F32R, name=f"xt{c}", tag=f"xt{c}")
            for c in range(NCC)
        ]

        # small attention epilogue tiles
        yts = top.enter_context(tc.tile_pool(name="yts", bufs=2))

        # out-projection weights: loaded once during startup
        woutp = top.enter_context(tc.tile_pool(name="woutp", bufs=1))

        # x staging, released after phase 1 (must be stack-top at release)
        xq = tc.alloc_tile_pool(name="xq", bufs=3)

        # DMA emission order: ident + first x tile first, so the PE can
        # start transposing while the bulk constant loads stream behind
        ident_sb = const.tile([128, 128], F32R, name="ident_sb")
        nc.sync.dma_start(out=ident_sb, in_=ident[:, :])
        xloads = [None] * NTT
        xloads[0] = xq.tile([128, C], F32R, name="xload0", tag="xload")
        nc.scalar.dma_start(out=xloads[0][:, 0:512], in_=x[0:128, 0:512])
        nc.sync.dma_start(out=xloads[0][:, 512:1024], in_=x[0:128, 512:1024])
        mask_sb = []
        for i, msrc in enumerate((maskp0, maskp1)):
            mt = const.tile([128, 1024], F32R, name=f"mask{i}", tag=f"mask{i}")
            nc.sync.dma_start(out=mt, in_=msrc[:, :])
            mask_sb.append(mt)
        bq_sb = const.tile([128, NFT], F32, name="bq_sb")
        nc.sync.dma_start(out=bq_sb, in_=bq[:, :])
        bk_sb = const.tile([128, NFT], F32, name="bk_sb")
        nc.sync.dma_start(out=bk_sb, in_=bk[:, :])
        ones_sb = const.tile([128, HL], F32, name="ones_sb")
        nc.gpsimd.memset(ones_sb, 1.0)
        wout_sb = [
            woutp.tile([128, C], F32R, name=f"wout{fc}", tag=f"wout{fc}")
            for fc in range(NFT)
        ]

        # ---- Phase 1: transpose x and project v (fused per token tile) ----
        with tc.tile_pool(name="wvp", bufs=1) as wvp, \
             tc.tile_pool(name="ph1ps", bufs=5, space="PSUM") as ph1ps, \
             tc.tile_pool(name="vps", bufs=3, space="PSUM") as vps:
            wv_sb = []
            for cc in range(NCC):
                wvt = wvp.tile([128, F], F32R, name=f"wv{cc}", tag=f"wv{cc}")
                nc.sync.dma_start(out=wvt, in_=wv[128 * cc:128 * (cc + 1), :])
                wv_sb.append(wvt)
            for tt in range(NTT):
                xload = xloads[tt]
                if tt + 1 < NTT:
                    xloads[tt + 1] = xq.tile(
                        [128, C], F32R, name=f"xload{tt + 1}", tag="xload"
                    )
                    nc.scalar.dma_start(
                        out=xloads[tt + 1],
                        in_=x[128 * (tt + 1):128 * (tt + 2), :],
                    )
                for cc in range(NCC):
                    xt_ps = ph1ps.tile([128, 128], F32R, name="xt_ps", tag="xt_ps")
                    nc.tensor.transpose(
                        xt_ps, xload[:, 128 * cc:128 * (cc + 1)], ident_sb
                    )
                    nc.vector.tensor_copy(xt[cc][:, 128 * tt:128 * (tt + 1)], xt_ps)
                v_ps = vps.tile([128, F], F32, name="v_ps", tag="v_ps")
                for cc in range(NCC):
                    nc.tensor.matmul(
                        v_ps,
                        lhsT=xt[cc][:, 128 * tt:128 * (tt + 1)],
                        rhs=wv_sb[cc],
                        start=(cc == 0),
                        stop=(cc == NCC - 1),
                    )
                vv = v_ch[tt].rearrange("p (h c) -> p h c", c=DA)
                nc.vector.tensor_copy(
                    vv[:, :, 0:D], v_ps.rearrange("p (h c) -> p h c", c=D)
                )
                nc.vector.tensor_copy(
                    vv[:, :, D:DA], ones_sb.rearrange("p (h o) -> p h o", o=1)
                )
        xq.release()

        # ---- Phase 2: per feature-tile: q/k projection + attention ----
        with tc.tile_pool(name="ph2", bufs=1) as ph2, \
             tc.tile_pool(name="wqkp", bufs=2) as wqkp, \
             tc.tile_pool(name="e2p", bufs=1) as e2p, \
             tc.tile_pool(name="ph2ps", bufs=2, space="PSUM") as ph2ps:
            for fc in range(NFT):
                nc.sync.dma_start(
                    out=wout_sb[fc], in_=wout[128 * fc:128 * (fc + 1), :]
                )
            pend = None

            def finish_block(b, yA, yB, hA, hB, pv_pair):
                # last score pair's PV closes both accumulation groups, then
                # each head's output block is normalized and staged to DRAM
                pv_pair(2 * b + 1, last=True)
                for y_ps, h in ((yA, hA), (yB, hB)):
                    recip_t = yts.tile([1, 512], F32, name="recip", tag="recip")
                    nc.vector.reciprocal(recip_t, y_ps[D:DA, :])
                    rb_t = yts.tile([64, 512], F32, name="rb", tag="rb")
                    nc.gpsimd.partition_broadcast(rb_t, recip_t[0:1, :])
                    yt_t = yts.tile([64, 512], F32R, name="yt", tag="yt")
                    nc.vector.tensor_mul(yt_t, y_ps[0:D, :], rb_t)
                    nc.sync.dma_start(
                        out=yt_dram[b][64 * h:64 * (h + 1), :],
                        in_=yt_t,
                    )

            def load_wqk(ft):
                wqf, wkf = [], []
                for cc in range(NCC):
                    wq_t = wqkp.tile(
                        [128, 128], F32R, name=f"wqf{cc}", tag=f"wqf{cc}"
                    )
                    nc.sync.dma_start(
                        out=wq_t,
                        in_=wq[128 * cc:128 * (cc + 1), 128 * ft:128 * (ft + 1)],
                    )
                    wqf.append(wq_t)
                    wk_t = wqkp.tile(
                        [128, 128], F32R, name=f"wkf{cc}", tag=f"wkf{cc}"
                    )
                    nc.sync.dma_start(
                        out=wk_t,
                        in_=wk[128 * cc:128 * (cc + 1), 128 * ft:128 * (ft + 1)],
                    )
                    wkf.append(wk_t)
                return wqf, wkf

            def emit_proj(ft, wqk):
                qft = ph2.tile([128, T], F32R, name="qft", tag="qft", bufs=1)
                kft = ph2.tile([128, T], F32R, name="kft", tag="kft", bufs=1)
                wqf, wkf = wqk
                for tb in range(NTB):
                    q_ps = ph2ps.tile(
                        [128, 512], F32, name="q_ps", tag="bank1", bufs=4
                    )
                    for cc in range(NCC):
                        nc.tensor.matmul(
                            q_ps,
                            lhsT=wqf[cc],
                            rhs=xt[cc][:, 512 * tb:512 * (tb + 1)],
                            start=(cc == 0),
                            stop=(cc == NCC - 1),
                        )
                    nc.vector.tensor_scalar_add(
                        qft[:, 512 * tb:512 * (tb + 1)], q_ps, bq_sb[:, ft:ft + 1]
                    )
                    k_ps = ph2ps.tile(
                        [128, 512], F32, name="k_ps", tag="bank1", bufs=4
                    )
                    for cc in range(NCC):
                        nc.tensor.matmul(
                            k_ps,
                            lhsT=wkf[cc],
                            rhs=xt[cc][:, 512 * tb:512 * (tb + 1)],
                            start=(cc == 0),
                            stop=(cc == NCC - 1),
                        )
                    nc.vector.tensor_scalar_add(
                        kft[:, 512 * tb:512 * (tb + 1)], k_ps, bk_sb[:, ft:ft + 1]
                    )
                return qft, kft

            wqk_next = load_wqk(0)
            for ft in range(NFT):
                qft, kft = emit_proj(ft, wqk_next)
                if ft + 1 < NFT:
                    wqk_next = load_wqk(ft + 1)

                # Both heads of this feature tile processed together: their
                # K=64 S^T matmuls sit in disjoint PE row groups (partition
                # bases 0 and 64), so adjacent emission lets the hardware
                # run each A/B pair concurrently. PV consumes score pairs
                # one pair behind the exp stream; the final pair's PV and
                # the normalization are deferred into the next block so the
                # PE never blocks on this block's activations.
                hA = 2 * ft
                hB = 2 * ft + 1
                for b in range(NTB):
                    npair = 2 * b + 2
                    yA = ph2ps.tile([DA, 512], F32, name="yA", tag="bank1", bufs=4)
                    yB = ph2ps.tile([DA, 512], F32, name="yB", tag="bank1", bufs=4)
                    eA, eB = {}, {}

                    def pv_pair(tp, last, _b=b, _yA=yA, _yB=yB, _eA=eA, _eB=eB,
                                _hA=hA, _hB=hB):
                        # default-arg binding: this closure is also called
                        # from the NEXT block via `pend`, after these names
                        # have been rebound
                        nt = 4 * _b + 4
                        for t in (2 * tp, 2 * tp + 1):
                            for y_ps, h, e in (
                                (_yA, _hA, _eA), (_yB, _hB, _eB)
                            ):
                                nc.tensor.matmul(
                                    y_ps,
                                    lhsT=v_ch[t][:, DA * h:DA * (h + 1)],
                                    rhs=e[tp][:, 512 * (t % 2):512 * (t % 2 + 1)],
                                    start=(t == 0),
                                    stop=(t == nt - 1) and last,
                                    skip_group_check=True,
                                )

                    for tp in range(npair):
                        sA = ph2ps.tile([128, 1024], F32, name="sA", tag="sA", bufs=1)
                        sB = ph2ps.tile([128, 1024], F32, name="sB", tag="sB", bufs=1)
                        for half in range(2):
                            t = 2 * tp + half
                            for s_ps, po in ((sA, 0), (sB, 64)):
                                nc.tensor.matmul(
                                    s_ps[:, 512 * half:512 * (half + 1)],
                                    lhsT=kft[po:po + 64, 128 * t:128 * (t + 1)],
                                    rhs=qft[po:po + 64, 512 * b:512 * (b + 1)],
                                    start=True,
                                    stop=True,
                                )
                        eA[tp] = e2p.tile(
                            [128, 1024], F32R, name=f"e2a{tp % 4}",
                            tag=f"e2a{tp % 4}",
                        )
                        nc.scalar.activation(eA[tp], sA, AF.Exp, scale=SCALE)
                        eB[tp] = e2p.tile(
                            [128, 1024], F32R, name=f"e2b{tp % 4}",
                            tag=f"e2b{tp % 4}",
                        )
                        nc.scalar.activation(eB[tp], sB, AF.Exp, scale=SCALE)
                        if tp >= npair - 2:
                            mk = mask_sb[tp - (npair - 2)]
                            nc.gpsimd.tensor_mul(eA[tp], eA[tp], mk)
                            nc.gpsimd.tensor_mul(eB[tp], eB[tp], mk)
                        if tp == 0 and pend is not None:
                            finish_block(*pend)
                            pend = None
                        if tp >= 1:
                            pv_pair(tp - 1, last=False)
                    pend = (b, yA, yB, hA, hB, pv_pair)
            finish_block(*pend)

        # ---- Phase 3: partial out-projection ----
        with tc.tile_pool(name="ph3", bufs=2) as ph3, \
             tc.tile_pool(name="ph3ps", bufs=4, space="PSUM") as ph3ps:
            def load_ytl(g):
                # one [128, 512] load per feature chunk covers 4 token tiles
                # with 2KB-contiguous rows (good DMA descriptor efficiency)
                tiles = []
                for fc in range(NFT):
                    yl = ph3.tile(
                        [128, 512], F32R, name=f"ytl{fc}", tag=f"ytl{fc}"
                    )
                    nc.sync.dma_start(
                        out=yl,
                        in_=yt_dram[g][128 * fc:128 * (fc + 1), :],
                    )
                    tiles.append(yl)
                return tiles

            ytl = load_ytl(0)
            for g in range(NTB):
                ytl_next = load_ytl(g + 1) if g + 1 < NTB else None
                for ti in range(4):
                    tt = 4 * g + ti
                    for eb in range(2):
                        o_ps = ph3ps.tile([128, 512], F32, name="o_ps", tag="o_ps")
                        for fc in range(NFT):
                            nc.tensor.matmul(
                                o_ps,
                                lhsT=ytl[fc][:, 128 * ti:128 * (ti + 1)],
                                rhs=wout_sb[fc][:, 512 * eb:512 * (eb + 1)],
                                start=(fc == 0),
                                stop=(fc == NFT - 1),
                            )
                        osb = ph3.tile(
                            [128, 512], F32, name="osb", tag="osb", bufs=3
                        )
                        nc.vector.tensor_copy(osb, o_ps)
                        nc.scalar.dma_start(
                            out=outp[
                                128 * tt:128 * (tt + 1), 512 * eb:512 * (eb + 1)
                            ],
                            in_=osb,
                        )
                ytl = ytl_next

    nc.finalize()
    return nc


def _host_constants():
    ident = np.eye(128, dtype=np.float32)
    # mask_k[jj, ii] = 1 if (128k + jj) <= ii  (keep j <= i within diagonal blk)
    masks = []
    jj = np.arange(128)[:, None]
    ii = np.arange(512)[None, :]
    for k in range(4):
        masks.append(((128 * k + jj) <= ii).astype(np.float32))
    maskp0 = np.concatenate([masks[0], masks[1]], axis=1)
    maskp1 = np.concatenate([masks[2], masks[3]], axis=1)
    return ident, np.ascontiguousarray(maskp0), np.ascontiguousarray(maskp1)


def _in_maps(x, w_qkv, b_qkv, w_out):
    ident, maskp0, maskp1 = _host_constants()
    maps = []
    for core in range(8):
        b, g = core // 2, core % 2
        cols = slice(512 * g, 512 * (g + 1))
        wq = np.ascontiguousarray(w_qkv[:, 0 * C:1 * C][:, cols])
        wk = np.ascontiguousarray(w_qkv[:, 1 * C:2 * C][:, cols])
        wv = np.ascontiguousarray(w_qkv[:, 2 * C:3 * C][:, cols])
        bq = np.ascontiguousarray(
            b_qkv[0 * C:1 * C][cols].reshape(NFT, 128).T
        )
        bk = np.ascontiguousarray(
            b_qkv[1 * C:2 * C][cols].reshape(NFT, 128).T
        )
        wo = np.ascontiguousarray(w_out[cols, :])
        maps.append(
            {
                "x": np.ascontiguousarray(x[b]),
                "wq": wq,
                "wk": wk,
                "wv": wv,
                "wout": wo,
                "bq": bq,
                "bk": bk,
                "ident": ident,
                "maskp0": maskp0,
                "maskp1": maskp1,
            }
        )
    return maps


_NC_CACHE = {}


def _get_nc():
    if "nc" not in _NC_CACHE:
        _NC_CACHE["nc"] = build_nc()
    return _NC_CACHE["nc"]


def run(inputs, trace=False, **spmd_kwargs):
    """Returns (output, BassKernelResults)."""
    x = np.asarray(inputs["x"], dtype=np.float32)
    w_qkv = np.asarray(inputs["w_qkv"], dtype=np.float32)
    b_qkv = np.asarray(inputs["b_qkv"], dtype=np.float32)
    w_out = np.asarray(inputs["w_out"], dtype=np.float32)
    b_out = np.asarray(inputs["b_out"], dtype=np.float32)

    nc = _get_nc()
    maps = _in_maps(x, w_qkv, b_qkv, w_out)
    res = run_bass_kernel_spmd(
        nc, maps, list(range(8)), trace=trace, **spmd_kwargs
    )
    out = np.empty((B, T, C), dtype=np.float32)
    for b in range(B):
        out[b] = res.results[2 * b]["out"] + res.results[2 * b + 1]["out"]
    # softmax rows sum to 1, so v-bias passes through attention unchanged:
    # its contribution to the output is b_v @ w_out, added once on the host.
    bias = b_out + b_qkv[2 * C:3 * C] @ w_out
    out += bias[None, None, :]
    return out, res


def kernel(**inputs):
    out, _ = run(inputs, trace=False)
    return out


# revision 56
# speedup vs baseline: 327.3120x; 1.0064x over previous
"""Causal self-attention Trainium2 kernel.

Problem: x[4,2048,1024] -> qkv proj -> 16-head causal attention -> out proj.

Sharding (8 cores): core = 2*batch + head_half. Each core handles one batch
(T=2048 tokens) and 8 of the 16 heads:
  - computes q^T,k^T (feature-major) and v (token-major) for its heads
  - S^T[j,i] = k^T.T-free attention scores, exp (no max-subtraction: logits
    are O(5) std-normal so exp is safe in fp32), causal mask, P^T @ v via
    an appended ones-column that yields the softmax row-sums for free
  - normalizes y, then computes the partial out-projection for its 512
    feature rows of w_out.
Host sums the two half-head partials per batch and adds biases. b_v is folded
in on the host via b_v @ w_out (exact since softmax rows sum to 1); b_out is
added on the host too.

All matmuls run as float32r (full-rate PE mode, fp32 storage).
"""

import numpy as np
from contextlib import ExitStack

import concourse.bass as bass
from concourse import bacc, mybir, tile
from concourse.bass_utils import run_bass_kernel_spmd

F32 = mybir.dt.float32
F32R = mybir.dt.float32r
AF = mybir.ActivationFunctionType

B = 4
T = 2048
C = 1024
H = 16
D = 64
SCALE = 1.0 / np.sqrt(D)

HL = 8            # heads per core
F = HL * D        # 512 local feature columns
NCC = C // 128    # 8 contraction chunks
NFT = F // 128    # 4 feature tiles (2 heads each)
NTT = T // 128    # 16 token tiles
NTB = T // 512    # 4 token blocks
DA = D + 1        # head dim + ones column


def build_nc():
    nc = bacc.Bacc()

    x = nc.declare_dram_parameter("x", [T, C], F32R, isOutput=False)
    wq = nc.declare_dram_parameter("wq", [C, F], F32R, isOutput=False)
    wk = nc.declare_dram_parameter("wk", [C, F], F32R, isOutput=False)
    wv = nc.declare_dram_parameter("wv", [C, F], F32R, isOutput=False)
    wout = nc.declare_dram_parameter("wout", [F, C], F32R, isOutput=False)
    bq = nc.declare_dram_parameter("bq", [128, NFT], F32, isOutput=False)
    bk = nc.declare_dram_parameter("bk", [128, NFT], F32, isOutput=False)
    ident = nc.declare_dram_parameter("ident", [128, 128], F32R, isOutput=False)
    maskp0 = nc.declare_dram_parameter("maskp0", [128, 1024], F32R, isOutput=False)
    maskp1 = nc.declare_dram_parameter("maskp1", [128, 1024], F32R, isOutput=False)
    outp = nc.declare_dram_parameter("out", [T, C], F32, isOutput=True)

    with tile.TileContext(nc) as tc, ExitStack() as top:
        const = top.enter_context(tc.tile_pool(name="const", bufs=1))
        dram = top.enter_context(tc.tile_pool(name="dram", bufs=1, space="DRAM"))
        yt_dram = [
            dram.tile([F, 512], F32R, name=f"yt_dram{g}", tag=f"yt{g}")
            for g in range(NTB)
        ]

        # v with an appended ones column per head: [token, 8*(64+1)]
        vaug = top.enter_context(tc.tile_pool(name="vaug", bufs=1))
        v_ch = [
            vaug.tile([128, HL * DA], F32R, name=f"v{t}", tag=f"v{t}")
            for t in range(NTT)
        ]

        # x^T, contraction dim on partitions: 8 chunks of [128, T]
        xt_pool = top.enter_context(tc.tile_pool(name="xt", bufs=1))
        xt = [
            xt_pool.tile([128, T], F32R, name=f"xt{c}", tag=f"xt{c}")
            for c in range(NCC)
        ]

        # small attention epilogue tiles
        yts = top.enter_context(tc.tile_pool(name="yts", bufs=2))

        # out-projection weights: loaded once during startup
        woutp = top.enter_context(tc.tile_pool(name="woutp", bufs=1))

        # x staging, released after phase 1 (must be stack-top at release)
        xq = tc.alloc_tile_pool(name="xq", bufs=3)

        # DMA emission order: ident + first x tile first, so the PE can
        # start transposing while the bulk constant loads stream behind
        ident_sb = const.tile([128, 128], F32R, name="ident_sb")
        nc.sync.dma_start(out=ident_sb, in_=ident[:, :])
        xloads = [None] * NTT
        xloads[0] = xq.tile([128, C], F32R, name="xload0", tag="xload")
        nc.scalar.dma_start(out=xloads[0][:, 0:512], in_=x[0:128, 0:512])
        nc.sync.dma_start(out=xloads[0][:, 512:1024], in_=x[0:128, 512:1024])
        mask_sb = []
        for i, msrc in enumerate((maskp0, maskp1)):
            mt = const.tile([128, 1024], F32R, name=f"mask{i}", tag=f"mask{i}")
            nc.sync.dma_start(out=mt, in_=msrc[:, :])
            mask_sb.append(mt)
        bq_sb = const.tile([128, NFT], F32, name="bq_sb")
        nc.sync.dma_start(out=bq_sb, in_=bq[:, :])
        bk_sb = const.tile([128, NFT], F32, name="bk_sb")
        nc.sync.dma_start(out=bk_sb, in_=bk[:, :])
        ones_sb = const.tile([128, HL], F32, name="ones_sb")
        nc.gpsimd.memset(ones_sb, 1.0)
        wout_sb = [
            woutp.tile([128, C], F32R, name=f"wout{fc}", tag=f"wout{fc}")
            for fc in range(NFT)
        ]

        # ---- Phase 1: transpose x and project v (fused per token tile) ----
        with tc.tile_pool(name="wvp", bufs=1) as wvp, \
             tc.tile_pool(name="ph1ps", bufs=5, space="PSUM") as ph1ps, \
             tc.tile_pool(name="vps", bufs=3, space="PSUM") as vps:
            wv_sb = []
            for cc in range(NCC):
                wvt = wvp.tile([128, F], F32R, name=f"wv{cc}", tag=f"wv{cc}")
                nc.sync.dma_start(out=wvt, in_=wv[128 * cc:128 * (cc + 1), :])
                wv_sb.append(wvt)
            for tt in range(NTT):
                xload = xloads[tt]
                if tt + 1 < NTT:
                    xloads[tt + 1] = xq.tile(
                        [128, C], F32R, name=f"xload{tt + 1}", tag="xload"
                    )
                    nc.scalar.dma_start(
                        out=xloads[tt + 1],
                        in_=x[128 * (tt + 1):128 * (tt + 2), :],
                    )
                for cc in range(NCC):
                    xt_ps = ph1ps.tile([128, 128], F32R, name="xt_ps", tag="xt_ps")
                    nc.tensor.transpose(
                        xt_ps, xload[:, 128 * cc:128 * (cc + 1)], ident_sb
                    )
                    nc.vector.tensor_copy(xt[cc][:, 128 * tt:128 * (tt + 1)], xt_ps)
                v_ps = vps.tile([128, F], F32, name="v_ps", tag="v_ps")
                for cc in range(NCC):
                    nc.tensor.matmul(
                        v_ps,
                        lhsT=xt[cc][:, 128 * tt:128 * (tt + 1)],
                        rhs=wv_sb[cc],
                        start=(cc == 0),
                        stop=(cc == NCC - 1),
                    )
                vv = v_ch[tt].rearrange("p (h c) -> p h c", c=DA)
                nc.vector.tensor_copy(
                    vv[:, :, 0:D], v_ps.rearrange("p (h c) -> p h c", c=D)
                )
                nc.vector.tensor_copy(
                    vv[:, :, D:DA], ones_sb.rearrange("p (h o) -> p h o", o=1)
                )
        xq.release()

        # ---- Phase 2: per feature-tile: q/k projection + attention ----
        with tc.tile_pool(name="ph2", bufs=1) as ph2, \
             tc.tile_pool(name="wqkp", bufs=2) as wqkp, \
             tc.tile_pool(name="e2p", bufs=1) as e2p, \
             tc.tile_pool(name="ph2ps", bufs=2, space="PSUM") as ph2ps:
            for fc in range(NFT):
                nc.sync.dma_start(
                    out=wout_sb[fc], in_=wout[128 * fc:128 * (fc + 1), :]
                )
            pend = None

            def finish_block(b, yA, yB, hA, hB, pv_pair):
                # the two deferred score pairs' PV close both accumulation
                # groups, then each head's output block is normalized and
                # staged to DRAM
                if 2 * b + 2 >= 2:
                    pv_pair(2 * b, last=False)
                pv_pair(2 * b + 1, last=True)
                for y_ps, h in ((yA, hA), (yB, hB)):
                    recip_t = yts.tile([1, 512], F32, name="recip", tag="recip")
                    nc.vector.reciprocal(recip_t, y_ps[D:DA, :])
                    rb_t = yts.tile([64, 512], F32, name="rb", tag="rb")
                    nc.gpsimd.partition_broadcast(rb_t, recip_t[0:1, :])
                    yt_t = yts.tile([64, 512], F32R, name="yt", tag="yt")
                    nc.vector.tensor_mul(yt_t, y_ps[0:D, :], rb_t)
                    nc.sync.dma_start(
                        out=yt_dram[b][64 * h:64 * (h + 1), :],
                        in_=yt_t,
                    )

            def load_wqk(ft):
                wqf, wkf = [], []
                for cc in range(NCC):
                    wq_t = wqkp.tile(
                        [128, 128], F32R, name=f"wqf{cc}", tag=f"wqf{cc}"
                    )
                    nc.sync.dma_start(
                        out=wq_t,
                        in_=wq[128 * cc:128 * (cc + 1), 128 * ft:128 * (ft + 1)],
                    )
                    wqf.append(wq_t)
                    wk_t = wqkp.tile(
                        [128, 128], F32R, name=f"wkf{cc}", tag=f"wkf{cc}"
                    )
                    nc.sync.dma_start(
                        out=wk_t,
                        in_=wk[128 * cc:128 * (cc + 1), 128 * ft:128 * (ft + 1)],
                    )
                    wkf.append(wk_t)
                return wqf, wkf

            def emit_proj(ft, wqk):
                qft = ph2.tile([128, T], F32R, name="qft", tag="qft", bufs=1)
                kft = ph2.tile([128, T], F32R, name="kft", tag="kft", bufs=1)
                wqf, wkf = wqk
                for tb in range(NTB):
                    q_ps = ph2ps.tile(
                        [128, 512], F32, name="q_ps", tag="bank1", bufs=4
                    )
                    for cc in range(NCC):
                        nc.tensor.matmul(
                            q_ps,
                            lhsT=wqf[cc],
                            rhs=xt[cc][:, 512 * tb:512 * (tb + 1)],
                            start=(cc == 0),
                            stop=(cc == NCC - 1),
                        )
                    nc.vector.tensor_scalar_add(
                        qft[:, 512 * tb:512 * (tb + 1)], q_ps, bq_sb[:, ft:ft + 1]
                    )
                    k_ps = ph2ps.tile(
                        [128, 512], F32, name="k_ps", tag="bank1", bufs=4
                    )
                    for cc in range(NCC):
                        nc.tensor.matmul(
                            k_ps,
                            lhsT=wkf[cc],
                            rhs=xt[cc][:, 512 * tb:512 * (tb + 1)],
                            start=(cc == 0),
                            stop=(cc == NCC - 1),
                        )
                    nc.vector.tensor_scalar_add(
                        kft[:, 512 * tb:512 * (tb + 1)], k_ps, bk_sb[:, ft:ft + 1]
                    )
                return qft, kft

            wqk_next = load_wqk(0)
            for ft in range(NFT):
                qft, kft = emit_proj(ft, wqk_next)
                if ft + 1 < NFT:
                    wqk_next = load_wqk(ft + 1)

                # Both heads of this feature tile processed together: their
                # K=64 S^T matmuls sit in disjoint PE row groups (partition
                # bases 0 and 64), so adjacent emission lets the hardware
                # run each A/B pair concurrently. PV consumes score pairs
                # one pair behind the exp stream; the final pair's PV and
                # the normalization are deferred into the next block so the
                # PE never blocks on this block's activations.
                hA = 2 * ft
                hB = 2 * ft + 1
                for b in range(NTB):
                    npair = 2 * b + 2
                    yA = ph2ps.tile([DA, 512], F32, name="yA", tag="bank1", bufs=4)
                    yB = ph2ps.tile([DA, 512], F32, name="yB", tag="bank1", bufs=4)
                    eA, eB = {}, {}

                    def pv_pair(tp, last, _b=b, _yA=yA, _yB=yB, _eA=eA, _eB=eB,
                                _hA=hA, _hB=hB):
                        # default-arg binding: this closure is also called
                        # from the NEXT block via `pend`, after these names
                        # have been rebound
                        nt = 4 * _b + 4
                        for t in (2 * tp, 2 * tp + 1):
                            for y_ps, h, e in (
                                (_yA, _hA, _eA), (_yB, _hB, _eB)
                            ):
                                nc.tensor.matmul(
                                    y_ps,
                                    lhsT=v_ch[t][:, DA * h:DA * (h + 1)],
                                    rhs=e[tp][:, 512 * (t % 2):512 * (t % 2 + 1)],
                                    start=(t == 0),
                                    stop=(t == nt - 1) and last,
                                    skip_group_check=True,
                                )

                    for tp in range(npair):
                        sA = ph2ps.tile([128, 1024], F32, name="sA", tag="sA", bufs=1)
                        sB = ph2ps.tile([128, 1024], F32, name="sB", tag="sB", bufs=1)
                        for half in range(2):
                            t = 2 * tp + half
                            for s_ps, po in ((sA, 0), (sB, 64)):
                                nc.tensor.matmul(
                                    s_ps[:, 512 * half:512 * (half + 1)],
                                    lhsT=kft[po:po + 64, 128 * t:128 * (t + 1)],
                                    rhs=qft[po:po + 64, 512 * b:512 * (b + 1)],
                                    start=True,
                                    stop=True,
                                )
                        eA[tp] = e2p.tile(
                            [128, 1024], F32R, name=f"e2a{tp % 4}",
                            tag=f"e2a{tp % 4}",
                        )
                        nc.scalar.activation(eA[tp], sA, AF.Exp, scale=SCALE)
                        eB[tp] = e2p.tile(
                            [128, 1024], F32R, name=f"e2b{tp % 4}",
                            tag=f"e2b{tp % 4}",
                        )
                        nc.scalar.activation(eB[tp], sB, AF.Exp, scale=SCALE)
                        if tp >= npair - 2:
                            mk = mask_sb[tp - (npair - 2)]
                            nc.gpsimd.tensor_mul(eA[tp], eA[tp], mk)
                            nc.gpsimd.tensor_mul(eB[tp], eB[tp], mk)
                        if tp == 0 and pend is not None:
                            finish_block(*pend)
                            pend = None
                        if tp >= 2:
                            pv_pair(tp - 2, last=False)
                    pend = (b, yA, yB, hA, hB, pv_pair)
            finish_block(*pend)

        # ---- Phase 3: partial out-projection ----
        with tc.tile_pool(name="ph3", bufs=2) as ph3, \
             tc.tile_pool(name="ph3ps", bufs=4, space="PSUM") as ph3ps:
            def load_ytl(g):
                # one [128, 512] load per feature chunk covers 4 token tiles
                # with 2KB-contiguous rows (good DMA descriptor efficiency)
                tiles = []
                for fc in range(NFT):
                    yl = ph3.tile(
                        [128, 512], F32R, name=f"ytl{fc}", tag=f"ytl{fc}"
                    )
                    nc.sync.dma_start(
                        out=yl,
                        in_=yt_dram[g][128 * fc:128 * (fc + 1), :],
                    )
                    tiles.append(yl)
                return tiles

            ytl = load_ytl(0)
            for g in range(NTB):
                ytl_next = load_ytl(g + 1) if g + 1 < NTB else None
                for ti in range(4):
                    tt = 4 * g + ti
                    for eb in range(2):
                        o_ps = ph3ps.tile([128, 512], F32, name="o_ps", tag="o_ps")
                        for fc in range(NFT):
                            nc.tensor.matmul(
                                o_ps,
                                lhsT=ytl[fc][:, 128 * ti:128 * (ti + 1)],
                                rhs=wout_sb[fc][:, 512 * eb:512 * (eb + 1)],
                                start=(fc == 0),
                                stop=(fc == NFT - 1),
                            )
                        osb = ph3.tile(
                            [128, 512], F32, name="osb", tag="osb", bufs=3
                        )
                        nc.vector.tensor_copy(osb, o_ps)
                        nc.scalar.dma_start(
                            out=outp[
                                128 * tt:128 * (tt + 1), 512 * eb:512 * (eb + 1)
                            ],
                            in_=osb,
                        )
                ytl = ytl_next

    nc.finalize()
    return nc


def _host_constants():
    ident = np.eye(128, dtype=np.float32)
    # mask_k[jj, ii] = 1 if (128k + jj) <= ii  (keep j <= i within diagonal blk)
    masks = []
    jj = np.arange(128)[:, None]
    ii = np.arange(512)[None, :]
    for k in range(4):
        masks.append(((128 * k + jj) <= ii).astype(np.float32))
    maskp0 = np.concatenate([masks[0], masks[1]], axis=1)
    maskp1 = np.concatenate([masks[2], masks[3]], axis=1)
    return ident, np.ascontiguousarray(maskp0), np.ascontiguousarray(maskp1)


def _in_maps(x, w_qkv, b_qkv, w_out):
    ident, maskp0, maskp1 = _host_constants()
    maps = []
    for core in range(8):
        b, g = core // 2, core % 2
        cols = slice(512 * g, 512 * (g + 1))
        wq = np.ascontiguousarray(w_qkv[:, 0 * C:1 * C][:, cols])
        wk = np.ascontiguousarray(w_qkv[:, 1 * C:2 * C][:, cols])
        wv = np.ascontiguousarray(w_qkv[:, 2 * C:3 * C][:, cols])
        bq = np.ascontiguousarray(
            b_qkv[0 * C:1 * C][cols].reshape(NFT, 128).T
        )
        bk = np.ascontiguousarray(
            b_qkv[1 * C:2 * C][cols].reshape(NFT, 128).T
        )
        wo = np.ascontiguousarray(w_out[cols, :])
        maps.append(
            {
                "x": np.ascontiguousarray(x[b]),
                "wq": wq,
                "wk": wk,
                "wv": wv,
                "wout": wo,
                "bq": bq,
                "bk": bk,
                "ident": ident,
                "maskp0": maskp0,
                "maskp1": maskp1,
            }
        )
    return maps


_NC_CACHE = {}


def _get_nc():
    if "nc" not in _NC_CACHE:
        _NC_CACHE["nc"] = build_nc()
    return _NC_CACHE["nc"]


def run(inputs, trace=False, **spmd_kwargs):
    """Returns (output, BassKernelResults)."""
    x = np.asarray(inputs["x"], dtype=np.float32)
    w_qkv = np.asarray(inputs["w_qkv"], dtype=np.float32)
    b_qkv = np.asarray(inputs["b_qkv"], dtype=np.float32)
    w_out = np.asarray(inputs["w_out"], dtype=np.float32)
    b_out = np.asarray(inputs["b_out"], dtype=np.float32)

    nc = _get_nc()
    maps = _in_maps(x, w_qkv, b_qkv, w_out)
    res = run_bass_kernel_spmd(
        nc, maps, list(range(8)), trace=trace, **spmd_kwargs
    )
    out = np.empty((B, T, C), dtype=np.float32)
    for b in range(B):
        out[b] = res.results[2 * b]["out"] + res.results[2 * b + 1]["out"]
    # softmax rows sum to 1, so v-bias passes through attention unchanged:
    # its contribution to the output is b_v @ w_out, added once on the host.
    bias = b_out + b_qkv[2 * C:3 * C] @ w_out
    out += bias[None, None, :]
    return out, res


def kernel(**inputs):
    out, _ = run(inputs, trace=False)
    return out
